# revision 24
# baseline (speedup 1.0000x reference)
"""Trainium2 Bass kernel for nn_DeepConvGraphEncoderPre.

Model: 4x GCN (dense normalized adjacency) -> mean-pool over nodes ->
single-step BiLSTM -> fc -> temporal attention over T -> linear head.

Sharding: data-parallel over batch B=8 across 8 NeuronCores (1 batch row
per core).  The normalized dense adjacency A^T [256,512-layout] is built
ON DEVICE from edge_index via one-hot matmuls (exact, handles duplicate
edges); self-loops are added analytically as an identity.  Every GCN
layer is two dense matmuls (aggregate-first): x <- relu((A x) W + b).

Key optimizations vs the f32r baseline:
- all GCN matmuls in bf16 (validated: final rel err ~3e-3 vs 2e-2 tol);
  every matmul streams at 1 cycle/row regardless of moving-free size.
- graph PAIRS merged into single matmuls for L1/L2 via block-diagonal
  W1/W2 (built on host), halving matmul count there.
- all weights are pre-laid-out and pre-cast on HOST (pure relayout);
  input data is host-transposed into the exact SBUF layout so the big
  DMA is 128 contiguous 8KB lines instead of 8192 x 256B descriptors.
- PSUM evacuations balanced across vector+scalar; node-pooling fused
  into relu via tensor_tensor_reduce on vector.
- LSTM tail: forget gate dropped (unused at window_size=1), sigmoid
  computed from tanh (host-folded 1/2 scales) so one activation-table
  load covers i/o/g/c; attention bias dropped (softmax shift-invariant);
  weighted sum via fused multiply-accumulate instead of extra matmuls.
"""

import numpy as np
import ml_dtypes

B, T, N, F, E = 8, 32, 256, 64, 4096
H, EMB, OUT = 256, 256, 512
NCORES = 8
NPAIR = T // 2  # graph pairs per core

_CACHE = {}
RUN_KWARGS = {}   # test harness may set {"trace": True, ...}
LAST_RESULT = None


def _build(flags):
    import concourse.mybir as mybir
    import concourse.tile as tile
    from concourse import bacc
    from concourse.masks import make_identity

    dt = mybir.dt
    f32, f32r, bf16, i32 = dt.float32, dt.float32r, dt.bfloat16, dt.int32
    AF = mybir.ActivationFunctionType
    ALU = mybir.AluOpType

    gcn_bias, lstm_bias, fc_bias, out_bias = (
        flags["gcn_bias"], flags["lstm_bias"], flags["fc_bias"], flags["out_bias"])

    nc = bacc.Bacc("TRN2", target_bir_lowering=False, debug=False,
                   num_devices=NCORES)

    def rf(ap):
        return ap.bitcast(f32r)

    # ---------------- DRAM I/O (all host-prepped layouts) ----------------
    x0_d = nc.dram_tensor("x0", [128, NPAIR * 256], bf16, kind="ExternalInput")
    edge_d = nc.dram_tensor("edge_index", [2, E], i32, kind="ExternalInput")
    W1_d = nc.dram_tensor("W1blk", [128, 128], bf16, kind="ExternalInput")
    W2_d = nc.dram_tensor("W2blk", [128, 256], bf16, kind="ExternalInput")
    W3_d = nc.dram_tensor("W3p", [128, 256], bf16, kind="ExternalInput")
    W4_d = nc.dram_tensor("W4p", [128, 512], bf16, kind="ExternalInput")
    WihT_d = nc.dram_tensor("WihTp", [128, 3072], f32r, kind="ExternalInput")
    fcW_d = nc.dram_tensor("fcWp", [128, 1024], f32r, kind="ExternalInput")
    attnW_d = nc.dram_tensor("attnWp", [128, 2], f32r, kind="ExternalInput")
    outW_d = nc.dram_tensor("outWp", [128, 1024], f32r, kind="ExternalInput")
    if gcn_bias:
        bb1_d = nc.dram_tensor("bb1", [128, 128], f32, kind="ExternalInput")
        bb2_d = nc.dram_tensor("bb2", [128, 256], f32, kind="ExternalInput")
        bb3_d = nc.dram_tensor("bb3", [128, 256], f32, kind="ExternalInput")
        b4c_d = nc.dram_tensor("b4col", [128, 2], f32, kind="ExternalInput")
    if lstm_bias:
        bihT_d = nc.dram_tensor("bihT", [1, 1536], f32r, kind="ExternalInput")
    if fc_bias:
        fcb_d = nc.dram_tensor("fcb_col", [128, 2], f32, kind="ExternalInput")
    if out_bias:
        outb_d = nc.dram_tensor("outb_row", [1, 512], f32r, kind="ExternalInput")
    out_d = nc.dram_tensor("out", [1, OUT], f32, kind="ExternalOutput")

    with tile.TileContext(nc) as tc:
        with tc.tile_pool(name="const", bufs=1) as cp:
            AT_sb = cp.tile([128, 512], bf16)       # col k*256+d ; A^T[s,d], s=k*128+p
            x0_sb = cp.tile([128, NPAIR * 256], bf16)
            W1_sb = cp.tile([128, 128], bf16)
            W2_sb = cp.tile([128, 256], bf16)
            W3_sb = cp.tile([128, 256], bf16)
            W4_sb = cp.tile([128, 512], bf16)
            WihT_sb = cp.tile([128, 3072], f32r)    # col k*1536 + g'*512 + d*256 + h
            fcW_sb = cp.tile([128, 1024], f32r)     # col k*256 + m   (pre-scaled 0.5)
            attnW_sb = cp.tile([128, 2], f32r)
            outW_sb = cp.tile([128, 1024], f32r)    # col mo*512 + o
            pooledT_sb = cp.tile([128, 64], f32r)   # col mo*32 + t
            ident = cp.tile([128, 128], f32)
            ones_col = cp.tile([128, 1], f32)
            ones_row = cp.tile([1, 128], f32)
            ones2c = cp.tile([128, 256], f32)
            if gcn_bias:
                bb1_sb = cp.tile([128, 128], f32)
                bb2_sb = cp.tile([128, 256], f32)
                bb3_sb = cp.tile([128, 256], f32)
                b4c_sb = cp.tile([128, 2], f32)
            if lstm_bias or out_bias:
                ones_f = cp.tile([1, 32], f32)
                ones_r = cp.tile([1, 32], f32r)
            if lstm_bias:
                bihT_sb = cp.tile([1, 1536], f32r)
            if fc_bias:
                fcb_sb = cp.tile([128, 2], f32)
            if out_bias:
                outb_sb = cp.tile([1, 512], f32r)

            # ---- DMA issue: sync gets the critical-path tensors, scalar
            # the weights (parallel issue on two queues) ----
            eg_sb = cp.tile([128, 64], i32)   # col j<32: src ; col 32+j: dst
            nc.sync.dma_start(
                out=eg_sb[:].rearrange("p (two j) -> p two j", two=2),
                in_=edge_d.ap().rearrange("two (p j) -> p two j", p=128))
            nc.sync.dma_start(out=x0_sb[:], in_=x0_d.ap())
            nc.sync.dma_start(out=WihT_sb[:], in_=WihT_d.ap())
            nc.sync.dma_start(out=W1_sb[:], in_=W1_d.ap())
            nc.sync.dma_start(out=W2_sb[:], in_=W2_d.ap())
            nc.sync.dma_start(out=W3_sb[:], in_=W3_d.ap())
            nc.sync.dma_start(out=W4_sb[:], in_=W4_d.ap())
            nc.sync.dma_start(out=fcW_sb[:], in_=fcW_d.ap())
            nc.sync.dma_start(out=attnW_sb[:], in_=attnW_d.ap())
            nc.sync.dma_start(out=outW_sb[:], in_=outW_d.ap())
            if gcn_bias:
                nc.sync.dma_start(out=bb1_sb[:], in_=bb1_d.ap())
                nc.sync.dma_start(out=bb2_sb[:], in_=bb2_d.ap())
                nc.sync.dma_start(out=bb3_sb[:], in_=bb3_d.ap())
                nc.sync.dma_start(out=b4c_sb[:], in_=b4c_d.ap())
            if lstm_bias:
                nc.sync.dma_start(out=bihT_sb[:], in_=bihT_d.ap())
            if fc_bias:
                nc.sync.dma_start(out=fcb_sb[:], in_=fcb_d.ap())
            if out_bias:
                nc.sync.dma_start(out=outb_sb[:], in_=outb_d.ap())

            nc.gpsimd.memset(ones_col[:], 1.0)
            nc.gpsimd.memset(ones_row[:], 1.0)
            nc.gpsimd.memset(ones2c[:], 1.0)
            make_identity(nc, ident[:])
            if lstm_bias or out_bias:
                nc.gpsimd.memset(ones_f[:], 1.0)
                nc.vector.tensor_copy(ones_r[:], ones_f[:])

            # ============ stage A: A^T build ============
            with (
                tc.tile_pool(name="ab_sb", bufs=2) as ab,
                tc.tile_pool(name="ab_ps", bufs=1, space="PSUM") as abp,
                tc.tile_pool(name="oh", bufs=4) as ohp,
            ):
                # iota 0..255 on every partition (channel_multiplier=0), cast bf16
                iota_i = ab.tile([128, 256], i32)
                nc.gpsimd.iota(iota_i[:], pattern=[[1, 256]], base=0,
                               channel_multiplier=0)
                iota_bc = ab.tile([128, 256], bf16)
                nc.vector.tensor_copy(iota_bc[:], iota_i[:])
                eg_b = ab.tile([128, 64], f32)
                nc.vector.tensor_copy(eg_b[:], eg_sb[:])

                # accumulate A^T_unnorm = sum_e onehot_src^T(slice) @ onehot_dst
                # (src one-hots on vector, dst one-hots on gpsimd)
                atun_ps = abp.tile([128, 512], f32)
                for c in range(32):
                    oh_s = ohp.tile([128, 256], bf16, tag="ohs")
                    nc.vector.tensor_scalar(oh_s[:], iota_bc[:],
                                            eg_b[:, c:c + 1], None,
                                            op0=ALU.is_equal)
                    oh_d = ohp.tile([128, 256], bf16, tag="ohd")
                    nc.gpsimd.tensor_scalar(oh_d[:], iota_bc[:],
                                            eg_b[:, 32 + c:33 + c], None,
                                            op0=ALU.is_equal)
                    for m in (0, 1):
                        nc.tensor.matmul(atun_ps[:, m * 256:(m + 1) * 256],
                                         oh_s[:, m * 128:(m + 1) * 128], oh_d[:],
                                         start=(c == 0 and m == 0),
                                         stop=(c == 31 and m == 1))
                atun_sb = ab.tile([128, 512], f32r)
                nc.vector.tensor_copy(atun_sb[:], atun_ps[:])
                # self-loops: += I on the diagonal (s = m*128+p, d = s)
                for m in (0, 1):
                    sl = atun_sb[:, m * 256 + m * 128: m * 256 + (m + 1) * 128]
                    nc.vector.tensor_add(sl, sl, ident[:])

                # deg (row + col forms), dinv = 1/sqrt(deg)   (deg >= 1 always)
                deg_ps = abp.tile([1, 256], f32, tag="deg")
                for m in (0, 1):
                    nc.tensor.matmul(deg_ps[:], ones_col[:],
                                     atun_sb[:, m * 256:(m + 1) * 256].bitcast(f32),
                                     start=(m == 0), stop=(m == 1))
                degc_ps = abp.tile([128, 2], f32, tag="degc")
                for dm in (0, 1):
                    for m in (0, 1):
                        nc.tensor.matmul(
                            degc_ps[:, dm:dm + 1],
                            atun_sb[:, m * 256 + dm * 128:
                                    m * 256 + (dm + 1) * 128].bitcast(f32),
                            ones_col[:], start=(m == 0), stop=(m == 1))
                dinv_row = ab.tile([1, 256], f32)
                nc.vector.reciprocal(dinv_row[:], deg_ps[:])
                nc.scalar.sqrt(dinv_row[:], dinv_row[:])
                dinv_col = ab.tile([128, 2], f32)
                nc.vector.reciprocal(dinv_col[:], degc_ps[:])
                nc.scalar.sqrt(dinv_col[:], dinv_col[:])
                # broadcast dinv to all partitions via rank-1 PE matmul
                dinv_bc = abp.tile([128, 256], f32, tag="dbc")
                nc.tensor.matmul(dinv_bc[:], ones_row[:], dinv_row[:],
                                 start=True, stop=True)

                # AT[s,d] = dinv[s] * ATun[s,d] * dinv[d]  (bf16 out)
                for m in (0, 1):
                    nc.vector.scalar_tensor_tensor(
                        out=AT_sb[:, m * 256:(m + 1) * 256],
                        in0=atun_sb[:, m * 256:(m + 1) * 256],
                        scalar=dinv_col[:, m:m + 1],
                        in1=dinv_bc[:],
                        op0=ALU.mult, op1=ALU.mult)

            # ================= stage B: GCN loop (graph pairs) =================
            with (
                tc.tile_pool(name="work", bufs=2) as wk,
                tc.tile_pool(name="psA", bufs=2, space="PSUM") as psA,
                tc.tile_pool(name="psB", bufs=2, space="PSUM") as psB,
                tc.tile_pool(name="psC", bufs=2, space="PSUM") as psC,
            ):
                # Two pairs in flight (software pipelining): each stage is
                # emitted for both pairs back-to-back so the cross-engine
                # dependency latency of one pair hides under the other's work.
                tl_ = {}

                def s_l1a(j):
                    agg1 = psC.tile([128, 256], f32, tag="C")
                    tl_[j, "agg1"] = agg1
                    for k in (0, 1):
                        nc.tensor.matmul(
                            agg1[:],
                            x0_sb[:, j * 256 + k * 128: j * 256 + (k + 1) * 128],
                            AT_sb[:, k * 256:(k + 1) * 256],
                            start=(k == 0), stop=(k == 1))

                def s_l1ev(j):
                    agg1_sb = wk.tile([128, 256], bf16, tag="agg1")
                    tl_[j, "agg1_sb"] = agg1_sb
                    nc.vector.tensor_copy(agg1_sb[:], tl_[j, "agg1"][:])

                def s_l1w(j):
                    z1 = psC.tile([128, 256], f32, tag="C")
                    tl_[j, "z1"] = z1
                    agg1_sb = tl_[j, "agg1_sb"]
                    for m in (0, 1):
                        nc.tensor.matmul(z1[:, m * 128:(m + 1) * 128],
                                         agg1_sb[:, m * 128:(m + 1) * 128],
                                         W1_sb[:], start=True, stop=True)

                def s_x1(j):
                    z1 = tl_[j, "z1"]
                    x1 = wk.tile([128, 256], bf16, tag="x1")
                    tl_[j, "x1"] = x1
                    if gcn_bias:
                        z1b = wk.tile([128, 256], f32, tag="z1b")
                        nc.vector.tensor_add(
                            z1b[:].rearrange("p (m q) -> p m q", m=2),
                            z1[:].rearrange("p (m q) -> p m q", m=2),
                            bb1_sb[:].rearrange("p q -> p 1 q").broadcast_to([128, 2, 128]))
                        nc.scalar.activation(x1[:], z1b[:], AF.Relu)
                    else:
                        nc.scalar.activation(x1[:], z1[:], AF.Relu)

                def s_l2a(j):
                    agg2 = psC.tile([128, 256], f32, tag="C")
                    tl_[j, "agg2"] = agg2
                    x1 = tl_[j, "x1"]
                    for k in (0, 1):
                        nc.tensor.matmul(agg2[:],
                                         x1[:, k * 128:(k + 1) * 128],
                                         AT_sb[:, k * 256:(k + 1) * 256],
                                         start=(k == 0), stop=(k == 1))

                def s_l2ev(j):
                    agg2_sb = wk.tile([128, 256], bf16, tag="agg2")
                    tl_[j, "agg2_sb"] = agg2_sb
                    nc.vector.tensor_copy(agg2_sb[:], tl_[j, "agg2"][:])

                def s_l2w(j):
                    z2 = psB.tile([128, 512], f32, tag="B")
                    tl_[j, "z2"] = z2
                    agg2_sb = tl_[j, "agg2_sb"]
                    for m in (0, 1):
                        nc.tensor.matmul(z2[:, m * 256:(m + 1) * 256],
                                         agg2_sb[:, m * 128:(m + 1) * 128],
                                         W2_sb[:], start=True, stop=True)

                def s_x2(j):
                    z2 = tl_[j, "z2"]
                    x2 = wk.tile([128, 512], bf16, tag="x2")
                    tl_[j, "x2"] = x2
                    if gcn_bias:
                        z2b = wk.tile([128, 512], f32, tag="z2b")
                        nc.vector.tensor_add(
                            z2b[:].rearrange("p (m q) -> p m q", m=2),
                            z2[:].rearrange("p (m q) -> p m q", m=2),
                            bb2_sb[:].rearrange("p q -> p 1 q").broadcast_to([128, 2, 256]))
                        nc.scalar.activation(x2[:], z2b[:], AF.Relu)
                    else:
                        nc.scalar.activation(x2[:], z2[:], AF.Relu)

                def s_l3a(j):
                    agg3 = psB.tile([128, 512], f32, tag="B")
                    tl_[j, "agg3"] = agg3
                    x2 = tl_[j, "x2"]
                    for g in (0, 1):
                        for k in (0, 1):
                            nc.tensor.matmul(
                                agg3[:, g * 256:(g + 1) * 256],
                                x2[:, k * 256 + g * 128: k * 256 + (g + 1) * 128],
                                AT_sb[:, k * 256:(k + 1) * 256],
                                start=(k == 0), stop=(k == 1))

                def s_l3ev(j):
                    agg3_sb = wk.tile([128, 512], bf16, tag="agg3")
                    tl_[j, "agg3_sb"] = agg3_sb
                    nc.vector.tensor_copy(agg3_sb[:], tl_[j, "agg3"][:])

                def s_l3w(j):
                    z3 = psA.tile([128, 1024], f32, tag="A")
                    tl_[j, "z3"] = z3
                    agg3_sb = tl_[j, "agg3_sb"]
                    for g in (0, 1):
                        for m in (0, 1):
                            nc.tensor.matmul(
                                z3[:, g * 512 + m * 256: g * 512 + (m + 1) * 256],
                                agg3_sb[:, g * 256 + m * 128: g * 256 + (m + 1) * 128],
                                W3_sb[:], start=True, stop=True)

                def s_x3(j):
                    z3 = tl_[j, "z3"]
                    x3 = wk.tile([128, 1024], bf16, tag="x3")
                    tl_[j, "x3"] = x3
                    if gcn_bias:
                        z3b = wk.tile([128, 1024], f32, tag="z3b")
                        nc.vector.tensor_add(
                            z3b[:].rearrange("p (gm q) -> p gm q", gm=4),
                            z3[:].rearrange("p (gm q) -> p gm q", gm=4),
                            bb3_sb[:].rearrange("p q -> p 1 q").broadcast_to([128, 4, 256]))
                        nc.scalar.activation(x3[:], z3b[:], AF.Relu)
                    else:
                        nc.scalar.activation(x3[:], z3[:], AF.Relu)

                def s_l4a(j):
                    agg4 = psA.tile([128, 1024], f32, tag="A")
                    tl_[j, "agg4"] = agg4
                    x3 = tl_[j, "x3"]
                    for g in (0, 1):
                        for mc in (0, 1):
                            for k in (0, 1):
                                nc.tensor.matmul(
                                    agg4[:, g * 512 + mc * 256: g * 512 + (mc + 1) * 256],
                                    x3[:, g * 512 + k * 256 + mc * 128:
                                          g * 512 + k * 256 + (mc + 1) * 128],
                                    AT_sb[:, k * 256:(k + 1) * 256],
                                    start=(k == 0), stop=(k == 1))

                def s_l4ev(j):
                    # split the big evacuation across vector (g=0) and scalar
                    # (g=1) so neither engine eats the full 1024 columns
                    agg4 = tl_[j, "agg4"]
                    a4a = wk.tile([128, 512], bf16, tag="agg4a")
                    a4b = wk.tile([128, 512], bf16, tag="agg4b")
                    tl_[j, "a4"] = (a4a, a4b)
                    nc.vector.tensor_copy(a4a[:], agg4[:, 0:512])
                    nc.scalar.copy(a4b[:], agg4[:, 512:1024])

                def s_l4w(j):
                    z4 = psA.tile([128, 1024], f32, tag="A")
                    tl_[j, "z4"] = z4
                    a4 = tl_[j, "a4"]
                    for g in (0, 1):
                        for mo in (0, 1):
                            for k in (0, 1):
                                nc.tensor.matmul(
                                    z4[:, g * 512 + mo * 256: g * 512 + (mo + 1) * 256],
                                    W4_sb[:, k * 256 + mo * 128: k * 256 + (mo + 1) * 128],
                                    a4[g][:, k * 256:(k + 1) * 256],
                                    start=(k == 0), stop=(k == 1))

                def s_pool(j):
                    # fused relu + node-sum via STT accumulate (1/N in WihT)
                    z4 = tl_[j, "z4"]
                    x4 = wk.tile([128, 1024], bf16, tag="x4")
                    if gcn_bias:
                        for g in (0, 1):
                            for mo in (0, 1):
                                sl = slice(g * 512 + mo * 256, g * 512 + (mo + 1) * 256)
                                nc.scalar.activation(
                                    x4[:, sl], z4[:, sl], AF.Relu,
                                    bias=b4c_sb[:, mo:mo + 1])
                        with nc.allow_low_precision(reason="f32r pool accum"):
                            for g in (0, 1):
                                for mo in (0, 1):
                                    sl = slice(g * 512 + mo * 256,
                                               g * 512 + (mo + 1) * 256)
                                    nc.vector.tensor_reduce(
                                        out=pooledT_sb[:, mo * 32 + 2 * j + g:
                                                       mo * 32 + 2 * j + g + 1],
                                        in_=x4[:, sl],
                                        axis=mybir.AxisListType.X, op=ALU.add)
                    else:
                        with nc.allow_low_precision(reason="f32r pool accum"):
                            for g in (0, 1):
                                for mo in (0, 1):
                                    sl = slice(g * 512 + mo * 256,
                                               g * 512 + (mo + 1) * 256)
                                    nc.vector.scalar_tensor_tensor(
                                        out=x4[:, sl], in0=z4[:, sl], scalar=0.0,
                                        in1=ones2c[:], op0=ALU.max, op1=ALU.mult,
                                        accum_out=pooledT_sb[:, mo * 32 + 2 * j + g:
                                                             mo * 32 + 2 * j + g + 1])

                stages = [s_l1a, s_l1ev, s_l1w, s_x1, s_l2a, s_l2ev, s_l2w,
                          s_x2, s_l3a, s_l3ev, s_l3w, s_x3, s_l4a, s_l4ev,
                          s_l4w, s_pool]
                for base in range(0, NPAIR, 2):
                    for st in stages:
                        st(base)
                        st(base + 1)
                    tl_.clear()

            # ======= stage C: LSTM + fc + attention + head =======
            # gates laid out (i, o, g) x (dir) x h; i,o have 0.5 folded into
            # WihT so sigmoid(x) = 0.5*(1+tanh(x/2)) needs only tanh.
            with (
                tc.tile_pool(name="tail", bufs=1) as tl,
                tc.tile_pool(name="tailps_g", bufs=1, space="PSUM") as tpg,
                tc.tile_pool(name="tailps", bufs=2, space="PSUM") as tp,
            ):
                g_ps = tpg.tile([32, 1536], f32, tag="gates")
                for s in range(3):
                    for k in (0, 1):
                        nc.tensor.matmul(
                            g_ps[:, s * 512:(s + 1) * 512],
                            rf(pooledT_sb[:, k * 32:(k + 1) * 32]),
                            WihT_sb[:, k * 1536 + s * 512: k * 1536 + (s + 1) * 512],
                            start=(k == 0),
                            stop=(k == 1 and not lstm_bias))
                    if lstm_bias:
                        nc.tensor.matmul(g_ps[:, s * 512:(s + 1) * 512],
                                         ones_r[:],
                                         bihT_sb[:, s * 512:(s + 1) * 512],
                                         start=False, stop=True)
                th = tl.tile([32, 1536], f32)
                nc.scalar.activation(th[:], g_ps[:], AF.Tanh)
                # c2 = 2c = (1+tanh(i/2))*tanh(g) ; tc = tanh(c2 * 0.5)
                c2 = tl.tile([32, 512], f32)
                nc.vector.scalar_tensor_tensor(
                    out=c2[:], in0=th[:, 0:512], scalar=1.0,
                    in1=th[:, 1024:1536], op0=ALU.add, op1=ALU.mult)
                tc_sb = tl.tile([32, 512], f32)
                nc.scalar.activation(tc_sb[:], c2[:], AF.Tanh, scale=0.5)
                # h2 = 2h = (1+tanh(o/2))*tanh(c); the remaining 1/2 is folded
                # into fcW (pre-scaled 0.5 on host)
                h2 = tl.tile([32, 512], f32)
                nc.vector.scalar_tensor_tensor(
                    out=h2[:], in0=th[:, 512:1024], scalar=1.0,
                    in1=tc_sb[:], op0=ALU.add, op1=ALU.mult)

                # transpose h2 -> hT [128, (k,t)]
                hT_ps = tp.tile([128, 128], f32, tag="small")
                for k in range(4):
                    nc.tensor.transpose(hT_ps[:, k * 32:(k + 1) * 32],
                                        h2[:, k * 128:(k + 1) * 128],
                                        ident[0:32, 0:32])
                hT_sb = tl.tile([128, 128], f32r)
                nc.vector.tensor_copy(hT_sb[:], hT_ps[:])

                # embT [128, (mo,t)] = fcW^T @ hT
                embT_ps = tp.tile([128, 64], f32, tag="small")
                for mo in (0, 1):
                    for k in range(4):
                        nc.tensor.matmul(
                            embT_ps[:, mo * 32:(mo + 1) * 32],
                            fcW_sb[:, k * 256 + mo * 128: k * 256 + (mo + 1) * 128],
                            hT_sb[:, k * 32:(k + 1) * 32],
                            start=(k == 0), stop=(k == 3))
                embT_sb = tl.tile([128, 64], f32r)
                if fc_bias:
                    for mo in (0, 1):
                        nc.scalar.activation(embT_sb[:, mo * 32:(mo + 1) * 32],
                                             embT_ps[:, mo * 32:(mo + 1) * 32],
                                             AF.Identity,
                                             bias=fcb_sb[:, mo:mo + 1])
                else:
                    nc.vector.tensor_copy(embT_sb[:], embT_ps[:])

                # attention scores [1, 32]; attn_b dropped (softmax shift-inv);
                # no max-subtract (scores are O(1) by construction)
                sc_ps = tp.tile([1, 32], f32, tag="small")
                for mo in (0, 1):
                    nc.tensor.matmul(sc_ps[:], attnW_sb[:, mo:mo + 1],
                                     embT_sb[:, mo * 32:(mo + 1) * 32],
                                     start=(mo == 0), stop=(mo == 1))
                ex = tl.tile([1, 32], f32)
                ssum = tl.tile([1, 1], f32)
                nc.scalar.activation(ex[:], sc_ps[:], AF.Exp, accum_out=ssum[:])
                rs = tl.tile([1, 1], f32)
                nc.vector.reciprocal(rs[:], ssum[:])
                w_row = tl.tile([1, 32], f32)
                nc.vector.tensor_scalar_mul(w_row[:], ex[:], rs[:])
                w_bc = tp.tile([128, 32], f32, tag="wbc")
                nc.tensor.matmul(w_bc[:], ones_row[:], w_row[:],
                                 start=True, stop=True)

                # x_weighted[m] = sum_t embT[m,t] * w[t]  (fused mul+accum)
                xw_scr = tl.tile([128, 64], f32r)
                xw_col = tl.tile([128, 2], f32r)
                with nc.allow_low_precision(reason="f32r weighted-sum accum"):
                    for mo in (0, 1):
                        nc.vector.scalar_tensor_tensor(
                            out=xw_scr[:, mo * 32:(mo + 1) * 32],
                            in0=embT_sb[:, mo * 32:(mo + 1) * 32], scalar=1.0,
                            in1=w_bc[:], op0=ALU.mult, op1=ALU.mult,
                            accum_out=xw_col[:, mo:mo + 1])

                # head: out = xw @ out_W (+ out_b)
                fin_ps = tp.tile([1, 512], f32, tag="small")
                for mo in (0, 1):
                    nc.tensor.matmul(fin_ps[:], xw_col[:, mo:mo + 1],
                                     outW_sb[:, mo * 512:(mo + 1) * 512],
                                     start=(mo == 0),
                                     stop=(mo == 1 and not out_bias))
                if out_bias:
                    nc.tensor.matmul(fin_ps[:], ones_r[0:1, 0:1], outb_sb[:],
                                     start=False, stop=True)
                fin_sb = tl.tile([1, 512], f32)
                nc.vector.tensor_copy(fin_sb[:], fin_ps[:])
                nc.sync.dma_start(out=out_d.ap(), in_=fin_sb[:])

    nc.compile()
    return nc


def _get_nc(flags):
    key = tuple(sorted(flags.items()))
    if key not in _CACHE:
        _CACHE[key] = _build(flags)
    return _CACHE[key]


def kernel(**inputs):
    from concourse import bass_utils

    bf = ml_dtypes.bfloat16
    inp = {k: np.asarray(v) for k, v in inputs.items()}
    flags = {
        "gcn_bias": any(np.any(inp[f"b{i}"]) for i in (1, 2, 3, 4)),
        "lstm_bias": any(np.any(inp[k]) for k in
                         ("b_ih_f", "b_hh_f", "b_ih_b", "b_hh_b")),
        "fc_bias": bool(np.any(inp["fc_b"])),
        "out_bias": bool(np.any(inp["out_b"])),
    }
    nc = _get_nc(flags)

    f32 = np.float32
    W1 = inp["W1"].astype(f32)
    W2 = inp["W2"].astype(f32)
    W1blk = np.zeros((128, 128), f32)
    W1blk[:64, :64] = W1
    W1blk[64:, 64:] = W1
    W2blk = np.zeros((128, 256), f32)
    W2blk[:64, :128] = W2
    W2blk[64:, 128:] = W2
    W4p = inp["W4"].astype(f32).reshape(2, 128, 256).transpose(1, 0, 2).reshape(128, 512)

    # WihT: [co, g'*512 + d*256 + h], gate order (i, o, g); i,o scaled 0.5
    # (sigmoid-from-tanh), everything scaled 1/N (mean-pool folded in)
    M = np.zeros((256, 1536), f32)
    for di, dname in enumerate(("f", "b")):
        Wih = inp[f"W_ih_{dname}"].astype(f32)  # [4H, H] rows gate*256+h
        for gdst, (gsrc, sc) in enumerate([(0, 0.5), (3, 0.5), (2, 1.0)]):
            M[:, gdst * 512 + di * 256: gdst * 512 + (di + 1) * 256] = \
                Wih[gsrc * 256:(gsrc + 1) * 256, :].T * (sc / N)
    WihTp = M.reshape(2, 128, 1536).transpose(1, 0, 2).reshape(128, 3072)

    fcWp = (inp["fc_W"].astype(f32) * 0.5).reshape(4, 128, 256) \
        .transpose(1, 0, 2).reshape(128, 1024)
    attnWp = np.ascontiguousarray(inp["attn_W"].astype(f32).reshape(2, 128).T)
    outWp = inp["out_W"].astype(f32).reshape(2, 128, 512) \
        .transpose(1, 0, 2).reshape(128, 1024)

    base = {
        "edge_index": np.ascontiguousarray(inp["edge_index"].astype(np.int32)),
        "W1blk": np.ascontiguousarray(W1blk.astype(bf)),
        "W2blk": np.ascontiguousarray(W2blk.astype(bf)),
        "W3p": np.ascontiguousarray(inp["W3"].astype(f32).astype(bf)),
        "W4p": np.ascontiguousarray(W4p.astype(bf)),
        "WihTp": np.ascontiguousarray(WihTp),
        "fcWp": np.ascontiguousarray(fcWp),
        "attnWp": attnWp,
        "outWp": np.ascontiguousarray(outWp),
    }
    if flags["gcn_bias"]:
        b1 = inp["b1"].astype(f32)
        b2 = inp["b2"].astype(f32)
        b3 = inp["b3"].astype(f32)
        b4 = inp["b4"].astype(f32)
        base["bb1"] = np.ascontiguousarray(
            np.tile(np.concatenate([b1, b1]), (128, 1)))
        base["bb2"] = np.ascontiguousarray(
            np.tile(np.concatenate([b2, b2]), (128, 1)))
        base["bb3"] = np.ascontiguousarray(np.tile(b3, (128, 1)))
        base["b4col"] = np.ascontiguousarray(b4.reshape(2, 128).T)
    if flags["lstm_bias"]:
        bihT = np.zeros((1, 1536), f32)
        for di, dname in enumerate(("f", "b")):
            bsum = (inp[f"b_ih_{dname}"] + inp[f"b_hh_{dname}"]).astype(f32)
            for gdst, (gsrc, sc) in enumerate([(0, 0.5), (3, 0.5), (2, 1.0)]):
                bihT[0, gdst * 512 + di * 256: gdst * 512 + (di + 1) * 256] = \
                    bsum[gsrc * 256:(gsrc + 1) * 256] * sc
        base["bihT"] = bihT
    if flags["fc_bias"]:
        base["fcb_col"] = np.ascontiguousarray(
            inp["fc_b"].astype(f32).reshape(2, 128).T)
    if flags["out_bias"]:
        base["outb_row"] = np.ascontiguousarray(
            inp["out_b"].astype(f32).reshape(1, 512))

    # x0: [p, j*256 + k*128 + g*64 + c] = data[2j+g, k*128+p, c], bf16
    data = inp["data"].astype(f32)
    in_maps = []
    for c in range(NCORES):
        v = data[c].reshape(NPAIR, 2, 2, 128, F)          # [j, g, k, p, c]
        x0 = v.transpose(3, 0, 2, 1, 4).reshape(128, NPAIR * 256)
        in_maps.append(dict(base, x0=np.ascontiguousarray(x0.astype(bf))))

    global LAST_RESULT
    res = bass_utils.run_bass_kernel_spmd(nc, in_maps,
                                          core_ids=list(range(NCORES)),
                                          **RUN_KWARGS)
    LAST_RESULT = res
    return np.concatenate([r["out"] for r in res.results], axis=0)


if __name__ == "__main__":
    import reference
    inputs = {k: np.asarray(v) for k, v in reference.setup_inputs().items()}
    got = kernel(**inputs)
    print(got.shape, got.dtype)


# revision 25
# speedup vs baseline: 1.9343x; 1.9343x over previous
"""Trainium2 Bass kernel for nn_DeepConvGraphEncoderPre.

Model: 4x GCN (dense normalized adjacency) -> mean-pool over nodes ->
single-step BiLSTM -> fc -> temporal attention over T -> linear head.

Sharding: data-parallel over batch B=8 across 8 NeuronCores (1 batch row
per core).  The normalized dense adjacency A^T [256,512-layout] is built
ON DEVICE from edge_index via one-hot matmuls (exact, handles duplicate
edges); self-loops are added analytically as an identity.  Every GCN
layer is two dense matmuls (aggregate-first): x <- relu((A x) W + b).

Key optimizations vs the f32r baseline:
- all GCN matmuls in bf16 (validated: final rel err ~3e-3 vs 2e-2 tol);
  every matmul streams at 1 cycle/row regardless of moving-free size.
- graph PAIRS merged into single matmuls for L1/L2 via block-diagonal
  W1/W2 (built on host), halving matmul count there.
- all weights are pre-laid-out and pre-cast on HOST (pure relayout);
  input data is host-transposed into the exact SBUF layout so the big
  DMA is 128 contiguous 8KB lines instead of 8192 x 256B descriptors.
- PSUM evacuations balanced across vector+scalar; node-pooling fused
  into relu via tensor_tensor_reduce on vector.
- LSTM tail: forget gate dropped (unused at window_size=1), sigmoid
  computed from tanh (host-folded 1/2 scales) so one activation-table
  load covers i/o/g/c; attention bias dropped (softmax shift-invariant);
  weighted sum via fused multiply-accumulate instead of extra matmuls.
"""

import numpy as np
import ml_dtypes

B, T, N, F, E = 8, 32, 256, 64, 4096
H, EMB, OUT = 256, 256, 512
NCORES = 8
NPAIR = T // 2  # graph pairs per core

_CACHE = {}
RUN_KWARGS = {}   # test harness may set {"trace": True, ...}
LAST_RESULT = None


def _build(flags):
    import concourse.mybir as mybir
    import concourse.tile as tile
    from concourse import bacc
    from concourse.masks import make_identity

    dt = mybir.dt
    f32, f32r, bf16, i32 = dt.float32, dt.float32r, dt.bfloat16, dt.int32
    AF = mybir.ActivationFunctionType
    ALU = mybir.AluOpType

    gcn_bias, lstm_bias, fc_bias, out_bias = (
        flags["gcn_bias"], flags["lstm_bias"], flags["fc_bias"], flags["out_bias"])

    nc = bacc.Bacc("TRN2", target_bir_lowering=False, debug=False,
                   num_devices=NCORES)

    def rf(ap):
        return ap.bitcast(f32r)

    # ---------------- DRAM I/O (all host-prepped layouts) ----------------
    x0_d = nc.dram_tensor("x0", [128, NPAIR * 256], bf16, kind="ExternalInput")
    edge_d = nc.dram_tensor("edge_index", [2, E], i32, kind="ExternalInput")
    W1_d = nc.dram_tensor("W1blk", [128, 128], bf16, kind="ExternalInput")
    W2_d = nc.dram_tensor("W2blk", [128, 256], bf16, kind="ExternalInput")
    W3_d = nc.dram_tensor("W3p", [128, 256], bf16, kind="ExternalInput")
    W4_d = nc.dram_tensor("W4p", [128, 512], bf16, kind="ExternalInput")
    WihT_d = nc.dram_tensor("WihTp", [128, 3072], f32r, kind="ExternalInput")
    fcW_d = nc.dram_tensor("fcWp", [128, 1024], f32r, kind="ExternalInput")
    attnW_d = nc.dram_tensor("attnWp", [128, 2], f32r, kind="ExternalInput")
    outW_d = nc.dram_tensor("outWp", [128, 1024], f32r, kind="ExternalInput")
    if gcn_bias:
        bb1_d = nc.dram_tensor("bb1", [128, 128], f32, kind="ExternalInput")
        bb2_d = nc.dram_tensor("bb2", [128, 256], f32, kind="ExternalInput")
        bb3_d = nc.dram_tensor("bb3", [128, 256], f32, kind="ExternalInput")
        b4c_d = nc.dram_tensor("b4col", [128, 2], f32, kind="ExternalInput")
    if lstm_bias:
        bihT_d = nc.dram_tensor("bihT", [1, 1536], f32r, kind="ExternalInput")
    if fc_bias:
        fcb_d = nc.dram_tensor("fcb_col", [128, 2], f32, kind="ExternalInput")
    if out_bias:
        outb_d = nc.dram_tensor("outb_row", [1, 512], f32r, kind="ExternalInput")
    out_d = nc.dram_tensor("out", [1, OUT], f32, kind="ExternalOutput")

    with tile.TileContext(nc) as tc:
        with tc.tile_pool(name="const", bufs=1) as cp:
            AT_sb = cp.tile([128, 512], bf16)       # col k*256+d ; A^T[s,d], s=k*128+p
            x0_sb = cp.tile([128, NPAIR * 256], bf16)
            W1_sb = cp.tile([128, 128], bf16)
            W2_sb = cp.tile([128, 256], bf16)
            W3_sb = cp.tile([128, 256], bf16)
            W4_sb = cp.tile([128, 512], bf16)
            WihT_sb = cp.tile([128, 3072], f32r)    # col k*1536 + g'*512 + d*256 + h
            fcW_sb = cp.tile([128, 1024], f32r)     # col k*256 + m   (pre-scaled 0.5)
            attnW_sb = cp.tile([128, 2], f32r)
            outW_sb = cp.tile([128, 1024], f32r)    # col mo*512 + o
            pooledT_sb = cp.tile([128, 64], f32r)   # col mo*32 + t
            ident = cp.tile([128, 128], f32)
            ones_col = cp.tile([128, 1], f32)
            ones_row = cp.tile([1, 128], f32)
            ones2c = cp.tile([128, 256], f32)
            if gcn_bias:
                bb1_sb = cp.tile([128, 128], f32)
                bb2_sb = cp.tile([128, 256], f32)
                bb3_sb = cp.tile([128, 256], f32)
                b4c_sb = cp.tile([128, 2], f32)
            if lstm_bias or out_bias:
                ones_f = cp.tile([1, 32], f32)
                ones_r = cp.tile([1, 32], f32r)
            if lstm_bias:
                bihT_sb = cp.tile([1, 1536], f32r)
            if fc_bias:
                fcb_sb = cp.tile([128, 2], f32)
            if out_bias:
                outb_sb = cp.tile([1, 512], f32r)

            # ---- DMA issue: sync gets the critical-path tensors, scalar
            # the weights (parallel issue on two queues) ----
            eg_sb = cp.tile([128, 64], i32)   # col j<32: src ; col 32+j: dst
            nc.sync.dma_start(
                out=eg_sb[:].rearrange("p (two j) -> p two j", two=2),
                in_=edge_d.ap().rearrange("two (p j) -> p two j", p=128))
            nc.sync.dma_start(out=x0_sb[:], in_=x0_d.ap())
            nc.sync.dma_start(out=WihT_sb[:], in_=WihT_d.ap())
            nc.sync.dma_start(out=W1_sb[:], in_=W1_d.ap())
            nc.sync.dma_start(out=W2_sb[:], in_=W2_d.ap())
            nc.sync.dma_start(out=W3_sb[:], in_=W3_d.ap())
            nc.sync.dma_start(out=W4_sb[:], in_=W4_d.ap())
            nc.sync.dma_start(out=fcW_sb[:], in_=fcW_d.ap())
            nc.sync.dma_start(out=attnW_sb[:], in_=attnW_d.ap())
            nc.sync.dma_start(out=outW_sb[:], in_=outW_d.ap())
            if gcn_bias:
                nc.sync.dma_start(out=bb1_sb[:], in_=bb1_d.ap())
                nc.sync.dma_start(out=bb2_sb[:], in_=bb2_d.ap())
                nc.sync.dma_start(out=bb3_sb[:], in_=bb3_d.ap())
                nc.sync.dma_start(out=b4c_sb[:], in_=b4c_d.ap())
            if lstm_bias:
                nc.sync.dma_start(out=bihT_sb[:], in_=bihT_d.ap())
            if fc_bias:
                nc.sync.dma_start(out=fcb_sb[:], in_=fcb_d.ap())
            if out_bias:
                nc.sync.dma_start(out=outb_sb[:], in_=outb_d.ap())

            nc.gpsimd.memset(ones_col[:], 1.0)
            nc.gpsimd.memset(ones_row[:], 1.0)
            nc.gpsimd.memset(ones2c[:], 1.0)
            make_identity(nc, ident[:])
            if lstm_bias or out_bias:
                nc.gpsimd.memset(ones_f[:], 1.0)
                nc.vector.tensor_copy(ones_r[:], ones_f[:])

            # ============ stage A: A^T build ============
            with (
                tc.tile_pool(name="ab_sb", bufs=2) as ab,
                tc.tile_pool(name="ab_ps", bufs=1, space="PSUM") as abp,
                tc.tile_pool(name="oh", bufs=4) as ohp,
            ):
                # iota 0..255 on every partition (channel_multiplier=0), cast bf16
                iota_i = ab.tile([128, 256], i32)
                nc.gpsimd.iota(iota_i[:], pattern=[[1, 256]], base=0,
                               channel_multiplier=0)
                iota_bc = ab.tile([128, 256], bf16)
                nc.vector.tensor_copy(iota_bc[:], iota_i[:])
                eg_b = ab.tile([128, 64], f32)
                nc.vector.tensor_copy(eg_b[:], eg_sb[:])

                # accumulate A^T_unnorm = sum_e onehot_src^T(slice) @ onehot_dst
                # (src one-hots on vector, dst one-hots on gpsimd)
                atun_ps = abp.tile([128, 512], f32)
                for c in range(32):
                    oh_s = ohp.tile([128, 256], bf16, tag="ohs")
                    nc.vector.tensor_scalar(oh_s[:], iota_bc[:],
                                            eg_b[:, c:c + 1], None,
                                            op0=ALU.is_equal)
                    oh_d = ohp.tile([128, 256], bf16, tag="ohd")
                    nc.vector.tensor_scalar(oh_d[:], iota_bc[:],
                                            eg_b[:, 32 + c:33 + c], None,
                                            op0=ALU.is_equal)
                    for m in (0, 1):
                        nc.tensor.matmul(atun_ps[:, m * 256:(m + 1) * 256],
                                         oh_s[:, m * 128:(m + 1) * 128], oh_d[:],
                                         start=(c == 0 and m == 0),
                                         stop=(c == 31 and m == 1))
                atun_sb = ab.tile([128, 512], f32r)
                nc.vector.tensor_copy(atun_sb[:], atun_ps[:])
                # self-loops: += I on the diagonal (s = m*128+p, d = s)
                for m in (0, 1):
                    sl = atun_sb[:, m * 256 + m * 128: m * 256 + (m + 1) * 128]
                    nc.vector.tensor_add(sl, sl, ident[:])

                # deg (row + col forms), dinv = 1/sqrt(deg)   (deg >= 1 always)
                deg_ps = abp.tile([1, 256], f32, tag="deg")
                for m in (0, 1):
                    nc.tensor.matmul(deg_ps[:], ones_col[:],
                                     atun_sb[:, m * 256:(m + 1) * 256].bitcast(f32),
                                     start=(m == 0), stop=(m == 1))
                degc_ps = abp.tile([128, 2], f32, tag="degc")
                for dm in (0, 1):
                    for m in (0, 1):
                        nc.tensor.matmul(
                            degc_ps[:, dm:dm + 1],
                            atun_sb[:, m * 256 + dm * 128:
                                    m * 256 + (dm + 1) * 128].bitcast(f32),
                            ones_col[:], start=(m == 0), stop=(m == 1))
                dinv_row = ab.tile([1, 256], f32)
                nc.vector.reciprocal(dinv_row[:], deg_ps[:])
                nc.scalar.sqrt(dinv_row[:], dinv_row[:])
                dinv_col = ab.tile([128, 2], f32)
                nc.vector.reciprocal(dinv_col[:], degc_ps[:])
                nc.scalar.sqrt(dinv_col[:], dinv_col[:])
                # broadcast dinv to all partitions via rank-1 PE matmul
                dinv_bc = abp.tile([128, 256], f32, tag="dbc")
                nc.tensor.matmul(dinv_bc[:], ones_row[:], dinv_row[:],
                                 start=True, stop=True)

                # AT[s,d] = dinv[s] * ATun[s,d] * dinv[d]  (bf16 out)
                for m in (0, 1):
                    nc.vector.scalar_tensor_tensor(
                        out=AT_sb[:, m * 256:(m + 1) * 256],
                        in0=atun_sb[:, m * 256:(m + 1) * 256],
                        scalar=dinv_col[:, m:m + 1],
                        in1=dinv_bc[:],
                        op0=ALU.mult, op1=ALU.mult)

            # ================= stage B: GCN loop (graph pairs) =================
            with (
                tc.tile_pool(name="work", bufs=2) as wk,
                tc.tile_pool(name="psA", bufs=2, space="PSUM") as psA,
                tc.tile_pool(name="psB", bufs=2, space="PSUM") as psB,
                tc.tile_pool(name="psC", bufs=2, space="PSUM") as psC,
            ):
                # Two pairs in flight (software pipelining): each stage is
                # emitted for both pairs back-to-back so the cross-engine
                # dependency latency of one pair hides under the other's work.
                tl_ = {}

                def s_l1a(j):
                    agg1 = psC.tile([128, 256], f32, tag="C")
                    tl_[j, "agg1"] = agg1
                    for k in (0, 1):
                        nc.tensor.matmul(
                            agg1[:],
                            x0_sb[:, j * 256 + k * 128: j * 256 + (k + 1) * 128],
                            AT_sb[:, k * 256:(k + 1) * 256],
                            start=(k == 0), stop=(k == 1))

                def s_l1ev(j):
                    agg1_sb = wk.tile([128, 256], bf16, tag="agg1")
                    tl_[j, "agg1_sb"] = agg1_sb
                    nc.vector.tensor_copy(agg1_sb[:], tl_[j, "agg1"][:])

                def s_l1w(j):
                    z1 = psC.tile([128, 256], f32, tag="C")
                    tl_[j, "z1"] = z1
                    agg1_sb = tl_[j, "agg1_sb"]
                    for m in (0, 1):
                        nc.tensor.matmul(z1[:, m * 128:(m + 1) * 128],
                                         agg1_sb[:, m * 128:(m + 1) * 128],
                                         W1_sb[:], start=True, stop=True)

                def s_x1(j):
                    z1 = tl_[j, "z1"]
                    x1 = wk.tile([128, 256], bf16, tag="x1")
                    tl_[j, "x1"] = x1
                    if gcn_bias:
                        z1b = wk.tile([128, 256], f32, tag="z1b")
                        nc.vector.tensor_add(
                            z1b[:].rearrange("p (m q) -> p m q", m=2),
                            z1[:].rearrange("p (m q) -> p m q", m=2),
                            bb1_sb[:].rearrange("p q -> p 1 q").broadcast_to([128, 2, 128]))
                        nc.scalar.activation(x1[:], z1b[:], AF.Relu)
                    else:
                        nc.scalar.activation(x1[:], z1[:], AF.Relu)

                def s_l2a(j):
                    agg2 = psC.tile([128, 256], f32, tag="C")
                    tl_[j, "agg2"] = agg2
                    x1 = tl_[j, "x1"]
                    for k in (0, 1):
                        nc.tensor.matmul(agg2[:],
                                         x1[:, k * 128:(k + 1) * 128],
                                         AT_sb[:, k * 256:(k + 1) * 256],
                                         start=(k == 0), stop=(k == 1))

                def s_l2ev(j):
                    agg2_sb = wk.tile([128, 256], bf16, tag="agg2")
                    tl_[j, "agg2_sb"] = agg2_sb
                    nc.vector.tensor_copy(agg2_sb[:], tl_[j, "agg2"][:])

                def s_l2w(j):
                    z2 = psB.tile([128, 512], f32, tag="B")
                    tl_[j, "z2"] = z2
                    agg2_sb = tl_[j, "agg2_sb"]
                    for m in (0, 1):
                        nc.tensor.matmul(z2[:, m * 256:(m + 1) * 256],
                                         agg2_sb[:, m * 128:(m + 1) * 128],
                                         W2_sb[:], start=True, stop=True)

                def s_x2(j):
                    z2 = tl_[j, "z2"]
                    x2 = wk.tile([128, 512], bf16, tag="x2")
                    tl_[j, "x2"] = x2
                    if gcn_bias:
                        z2b = wk.tile([128, 512], f32, tag="z2b")
                        nc.vector.tensor_add(
                            z2b[:].rearrange("p (m q) -> p m q", m=2),
                            z2[:].rearrange("p (m q) -> p m q", m=2),
                            bb2_sb[:].rearrange("p q -> p 1 q").broadcast_to([128, 2, 256]))
                        nc.scalar.activation(x2[:], z2b[:], AF.Relu)
                    else:
                        nc.scalar.activation(x2[:], z2[:], AF.Relu)

                def s_l3a(j):
                    agg3 = psB.tile([128, 512], f32, tag="B")
                    tl_[j, "agg3"] = agg3
                    x2 = tl_[j, "x2"]
                    for g in (0, 1):
                        for k in (0, 1):
                            nc.tensor.matmul(
                                agg3[:, g * 256:(g + 1) * 256],
                                x2[:, k * 256 + g * 128: k * 256 + (g + 1) * 128],
                                AT_sb[:, k * 256:(k + 1) * 256],
                                start=(k == 0), stop=(k == 1))

                def s_l3ev(j):
                    agg3_sb = wk.tile([128, 512], bf16, tag="agg3")
                    tl_[j, "agg3_sb"] = agg3_sb
                    nc.vector.tensor_copy(agg3_sb[:], tl_[j, "agg3"][:])

                def s_l3w(j):
                    z3 = psA.tile([128, 1024], f32, tag="A")
                    tl_[j, "z3"] = z3
                    agg3_sb = tl_[j, "agg3_sb"]
                    for g in (0, 1):
                        for m in (0, 1):
                            nc.tensor.matmul(
                                z3[:, g * 512 + m * 256: g * 512 + (m + 1) * 256],
                                agg3_sb[:, g * 256 + m * 128: g * 256 + (m + 1) * 128],
                                W3_sb[:], start=True, stop=True)

                def s_x3(j):
                    z3 = tl_[j, "z3"]
                    x3 = wk.tile([128, 1024], bf16, tag="x3")
                    tl_[j, "x3"] = x3
                    if gcn_bias:
                        z3b = wk.tile([128, 1024], f32, tag="z3b")
                        nc.vector.tensor_add(
                            z3b[:].rearrange("p (gm q) -> p gm q", gm=4),
                            z3[:].rearrange("p (gm q) -> p gm q", gm=4),
                            bb3_sb[:].rearrange("p q -> p 1 q").broadcast_to([128, 4, 256]))
                        nc.scalar.activation(x3[:], z3b[:], AF.Relu)
                    else:
                        nc.scalar.activation(x3[:], z3[:], AF.Relu)

                def s_l4a(j):
                    agg4 = psA.tile([128, 1024], f32, tag="A")
                    tl_[j, "agg4"] = agg4
                    x3 = tl_[j, "x3"]
                    for g in (0, 1):
                        for mc in (0, 1):
                            for k in (0, 1):
                                nc.tensor.matmul(
                                    agg4[:, g * 512 + mc * 256: g * 512 + (mc + 1) * 256],
                                    x3[:, g * 512 + k * 256 + mc * 128:
                                          g * 512 + k * 256 + (mc + 1) * 128],
                                    AT_sb[:, k * 256:(k + 1) * 256],
                                    start=(k == 0), stop=(k == 1))

                def s_l4ev(j):
                    # split the big evacuation across vector (g=0) and scalar
                    # (g=1) so neither engine eats the full 1024 columns
                    agg4 = tl_[j, "agg4"]
                    a4a = wk.tile([128, 512], bf16, tag="agg4a")
                    a4b = wk.tile([128, 512], bf16, tag="agg4b")
                    tl_[j, "a4"] = (a4a, a4b)
                    nc.vector.tensor_copy(a4a[:], agg4[:, 0:512])
                    nc.scalar.copy(a4b[:], agg4[:, 512:1024])

                def s_l4w(j):
                    z4 = psA.tile([128, 1024], f32, tag="A")
                    tl_[j, "z4"] = z4
                    a4 = tl_[j, "a4"]
                    for g in (0, 1):
                        for mo in (0, 1):
                            for k in (0, 1):
                                nc.tensor.matmul(
                                    z4[:, g * 512 + mo * 256: g * 512 + (mo + 1) * 256],
                                    W4_sb[:, k * 256 + mo * 128: k * 256 + (mo + 1) * 128],
                                    a4[g][:, k * 256:(k + 1) * 256],
                                    start=(k == 0), stop=(k == 1))

                def s_pool(j):
                    # fused relu + node-sum via STT accumulate (1/N in WihT)
                    z4 = tl_[j, "z4"]
                    x4 = wk.tile([128, 1024], bf16, tag="x4")
                    if gcn_bias:
                        for g in (0, 1):
                            for mo in (0, 1):
                                sl = slice(g * 512 + mo * 256, g * 512 + (mo + 1) * 256)
                                nc.scalar.activation(
                                    x4[:, sl], z4[:, sl], AF.Relu,
                                    bias=b4c_sb[:, mo:mo + 1])
                        with nc.allow_low_precision(reason="f32r pool accum"):
                            for g in (0, 1):
                                for mo in (0, 1):
                                    sl = slice(g * 512 + mo * 256,
                                               g * 512 + (mo + 1) * 256)
                                    nc.vector.tensor_reduce(
                                        out=pooledT_sb[:, mo * 32 + 2 * j + g:
                                                       mo * 32 + 2 * j + g + 1],
                                        in_=x4[:, sl],
                                        axis=mybir.AxisListType.X, op=ALU.add)
                    else:
                        with nc.allow_low_precision(reason="f32r pool accum"):
                            for g in (0, 1):
                                for mo in (0, 1):
                                    sl = slice(g * 512 + mo * 256,
                                               g * 512 + (mo + 1) * 256)
                                    nc.vector.scalar_tensor_tensor(
                                        out=x4[:, sl], in0=z4[:, sl], scalar=0.0,
                                        in1=ones2c[:], op0=ALU.max, op1=ALU.mult,
                                        accum_out=pooledT_sb[:, mo * 32 + 2 * j + g:
                                                             mo * 32 + 2 * j + g + 1])

                stages = [s_l1a, s_l1ev, s_l1w, s_x1, s_l2a, s_l2ev, s_l2w,
                          s_x2, s_l3a, s_l3ev, s_l3w, s_x3, s_l4a, s_l4ev,
                          s_l4w, s_pool]
                for base in range(0, NPAIR, 2):
                    for st in stages:
                        st(base)
                        st(base + 1)
                    tl_.clear()

            # ======= stage C: LSTM + fc + attention + head =======
            # gates laid out (i, o, g) x (dir) x h; i,o have 0.5 folded into
            # WihT so sigmoid(x) = 0.5*(1+tanh(x/2)) needs only tanh.
            with (
                tc.tile_pool(name="tail", bufs=1) as tl,
                tc.tile_pool(name="tailps_g", bufs=1, space="PSUM") as tpg,
                tc.tile_pool(name="tailps", bufs=2, space="PSUM") as tp,
            ):
                g_ps = tpg.tile([32, 1536], f32, tag="gates")
                for s in range(3):
                    for k in (0, 1):
                        nc.tensor.matmul(
                            g_ps[:, s * 512:(s + 1) * 512],
                            rf(pooledT_sb[:, k * 32:(k + 1) * 32]),
                            WihT_sb[:, k * 1536 + s * 512: k * 1536 + (s + 1) * 512],
                            start=(k == 0),
                            stop=(k == 1 and not lstm_bias))
                    if lstm_bias:
                        nc.tensor.matmul(g_ps[:, s * 512:(s + 1) * 512],
                                         ones_r[:],
                                         bihT_sb[:, s * 512:(s + 1) * 512],
                                         start=False, stop=True)
                th = tl.tile([32, 1536], f32)
                nc.scalar.activation(th[:], g_ps[:], AF.Tanh)
                # c2 = 2c = (1+tanh(i/2))*tanh(g) ; tc = tanh(c2 * 0.5)
                c2 = tl.tile([32, 512], f32)
                nc.vector.scalar_tensor_tensor(
                    out=c2[:], in0=th[:, 0:512], scalar=1.0,
                    in1=th[:, 1024:1536], op0=ALU.add, op1=ALU.mult)
                tc_sb = tl.tile([32, 512], f32)
                nc.scalar.activation(tc_sb[:], c2[:], AF.Tanh, scale=0.5)
                # h2 = 2h = (1+tanh(o/2))*tanh(c); the remaining 1/2 is folded
                # into fcW (pre-scaled 0.5 on host)
                h2 = tl.tile([32, 512], f32)
                nc.vector.scalar_tensor_tensor(
                    out=h2[:], in0=th[:, 512:1024], scalar=1.0,
                    in1=tc_sb[:], op0=ALU.add, op1=ALU.mult)

                # transpose h2 -> hT [128, (k,t)]
                hT_ps = tp.tile([128, 128], f32, tag="small")
                for k in range(4):
                    nc.tensor.transpose(hT_ps[:, k * 32:(k + 1) * 32],
                                        h2[:, k * 128:(k + 1) * 128],
                                        ident[0:32, 0:32])
                hT_sb = tl.tile([128, 128], f32r)
                nc.vector.tensor_copy(hT_sb[:], hT_ps[:])

                # embT [128, (mo,t)] = fcW^T @ hT
                embT_ps = tp.tile([128, 64], f32, tag="small")
                for mo in (0, 1):
                    for k in range(4):
                        nc.tensor.matmul(
                            embT_ps[:, mo * 32:(mo + 1) * 32],
                            fcW_sb[:, k * 256 + mo * 128: k * 256 + (mo + 1) * 128],
                            hT_sb[:, k * 32:(k + 1) * 32],
                            start=(k == 0), stop=(k == 3))
                embT_sb = tl.tile([128, 64], f32r)
                if fc_bias:
                    for mo in (0, 1):
                        nc.scalar.activation(embT_sb[:, mo * 32:(mo + 1) * 32],
                                             embT_ps[:, mo * 32:(mo + 1) * 32],
                                             AF.Identity,
                                             bias=fcb_sb[:, mo:mo + 1])
                else:
                    nc.vector.tensor_copy(embT_sb[:], embT_ps[:])

                # attention scores [1, 32]; attn_b dropped (softmax shift-inv);
                # no max-subtract (scores are O(1) by construction)
                sc_ps = tp.tile([1, 32], f32, tag="small")
                for mo in (0, 1):
                    nc.tensor.matmul(sc_ps[:], attnW_sb[:, mo:mo + 1],
                                     embT_sb[:, mo * 32:(mo + 1) * 32],
                                     start=(mo == 0), stop=(mo == 1))
                ex = tl.tile([1, 32], f32)
                ssum = tl.tile([1, 1], f32)
                nc.scalar.activation(ex[:], sc_ps[:], AF.Exp, accum_out=ssum[:])
                rs = tl.tile([1, 1], f32)
                nc.vector.reciprocal(rs[:], ssum[:])
                w_row = tl.tile([1, 32], f32)
                nc.vector.tensor_scalar_mul(w_row[:], ex[:], rs[:])
                w_bc = tp.tile([128, 32], f32, tag="wbc")
                nc.tensor.matmul(w_bc[:], ones_row[:], w_row[:],
                                 start=True, stop=True)

                # x_weighted[m] = sum_t embT[m,t] * w[t]  (fused mul+accum)
                xw_scr = tl.tile([128, 64], f32r)
                xw_col = tl.tile([128, 2], f32r)
                with nc.allow_low_precision(reason="f32r weighted-sum accum"):
                    for mo in (0, 1):
                        nc.vector.scalar_tensor_tensor(
                            out=xw_scr[:, mo * 32:(mo + 1) * 32],
                            in0=embT_sb[:, mo * 32:(mo + 1) * 32], scalar=1.0,
                            in1=w_bc[:], op0=ALU.mult, op1=ALU.mult,
                            accum_out=xw_col[:, mo:mo + 1])

                # head: out = xw @ out_W (+ out_b)
                fin_ps = tp.tile([1, 512], f32, tag="small")
                for mo in (0, 1):
                    nc.tensor.matmul(fin_ps[:], xw_col[:, mo:mo + 1],
                                     outW_sb[:, mo * 512:(mo + 1) * 512],
                                     start=(mo == 0),
                                     stop=(mo == 1 and not out_bias))
                if out_bias:
                    nc.tensor.matmul(fin_ps[:], ones_r[0:1, 0:1], outb_sb[:],
                                     start=False, stop=True)
                fin_sb = tl.tile([1, 512], f32)
                nc.vector.tensor_copy(fin_sb[:], fin_ps[:])
                nc.sync.dma_start(out=out_d.ap(), in_=fin_sb[:])

    nc.compile()
    return nc


def _get_nc(flags):
    key = tuple(sorted(flags.items()))
    if key not in _CACHE:
        _CACHE[key] = _build(flags)
    return _CACHE[key]


def kernel(**inputs):
    from concourse import bass_utils

    bf = ml_dtypes.bfloat16
    inp = {k: np.asarray(v) for k, v in inputs.items()}
    flags = {
        "gcn_bias": any(np.any(inp[f"b{i}"]) for i in (1, 2, 3, 4)),
        "lstm_bias": any(np.any(inp[k]) for k in
                         ("b_ih_f", "b_hh_f", "b_ih_b", "b_hh_b")),
        "fc_bias": bool(np.any(inp["fc_b"])),
        "out_bias": bool(np.any(inp["out_b"])),
    }
    nc = _get_nc(flags)

    f32 = np.float32
    W1 = inp["W1"].astype(f32)
    W2 = inp["W2"].astype(f32)
    W1blk = np.zeros((128, 128), f32)
    W1blk[:64, :64] = W1
    W1blk[64:, 64:] = W1
    W2blk = np.zeros((128, 256), f32)
    W2blk[:64, :128] = W2
    W2blk[64:, 128:] = W2
    W4p = inp["W4"].astype(f32).reshape(2, 128, 256).transpose(1, 0, 2).reshape(128, 512)

    # WihT: [co, g'*512 + d*256 + h], gate order (i, o, g); i,o scaled 0.5
    # (sigmoid-from-tanh), everything scaled 1/N (mean-pool folded in)
    M = np.zeros((256, 1536), f32)
    for di, dname in enumerate(("f", "b")):
        Wih = inp[f"W_ih_{dname}"].astype(f32)  # [4H, H] rows gate*256+h
        for gdst, (gsrc, sc) in enumerate([(0, 0.5), (3, 0.5), (2, 1.0)]):
            M[:, gdst * 512 + di * 256: gdst * 512 + (di + 1) * 256] = \
                Wih[gsrc * 256:(gsrc + 1) * 256, :].T * (sc / N)
    WihTp = M.reshape(2, 128, 1536).transpose(1, 0, 2).reshape(128, 3072)

    fcWp = (inp["fc_W"].astype(f32) * 0.5).reshape(4, 128, 256) \
        .transpose(1, 0, 2).reshape(128, 1024)
    attnWp = np.ascontiguousarray(inp["attn_W"].astype(f32).reshape(2, 128).T)
    outWp = inp["out_W"].astype(f32).reshape(2, 128, 512) \
        .transpose(1, 0, 2).reshape(128, 1024)

    base = {
        "edge_index": np.ascontiguousarray(inp["edge_index"].astype(np.int32)),
        "W1blk": np.ascontiguousarray(W1blk.astype(bf)),
        "W2blk": np.ascontiguousarray(W2blk.astype(bf)),
        "W3p": np.ascontiguousarray(inp["W3"].astype(f32).astype(bf)),
        "W4p": np.ascontiguousarray(W4p.astype(bf)),
        "WihTp": np.ascontiguousarray(WihTp),
        "fcWp": np.ascontiguousarray(fcWp),
        "attnWp": attnWp,
        "outWp": np.ascontiguousarray(outWp),
    }
    if flags["gcn_bias"]:
        b1 = inp["b1"].astype(f32)
        b2 = inp["b2"].astype(f32)
        b3 = inp["b3"].astype(f32)
        b4 = inp["b4"].astype(f32)
        base["bb1"] = np.ascontiguousarray(
            np.tile(np.concatenate([b1, b1]), (128, 1)))
        base["bb2"] = np.ascontiguousarray(
            np.tile(np.concatenate([b2, b2]), (128, 1)))
        base["bb3"] = np.ascontiguousarray(np.tile(b3, (128, 1)))
        base["b4col"] = np.ascontiguousarray(b4.reshape(2, 128).T)
    if flags["lstm_bias"]:
        bihT = np.zeros((1, 1536), f32)
        for di, dname in enumerate(("f", "b")):
            bsum = (inp[f"b_ih_{dname}"] + inp[f"b_hh_{dname}"]).astype(f32)
            for gdst, (gsrc, sc) in enumerate([(0, 0.5), (3, 0.5), (2, 1.0)]):
                bihT[0, gdst * 512 + di * 256: gdst * 512 + (di + 1) * 256] = \
                    bsum[gsrc * 256:(gsrc + 1) * 256] * sc
        base["bihT"] = bihT
    if flags["fc_bias"]:
        base["fcb_col"] = np.ascontiguousarray(
            inp["fc_b"].astype(f32).reshape(2, 128).T)
    if flags["out_bias"]:
        base["outb_row"] = np.ascontiguousarray(
            inp["out_b"].astype(f32).reshape(1, 512))

    # x0: [p, j*256 + k*128 + g*64 + c] = data[2j+g, k*128+p, c], bf16
    data = inp["data"].astype(f32)
    in_maps = []
    for c in range(NCORES):
        v = data[c].reshape(NPAIR, 2, 2, 128, F)          # [j, g, k, p, c]
        x0 = v.transpose(3, 0, 2, 1, 4).reshape(128, NPAIR * 256)
        in_maps.append(dict(base, x0=np.ascontiguousarray(x0.astype(bf))))

    global LAST_RESULT
    res = bass_utils.run_bass_kernel_spmd(nc, in_maps,
                                          core_ids=list(range(NCORES)),
                                          **RUN_KWARGS)
    LAST_RESULT = res
    return np.concatenate([r["out"] for r in res.results], axis=0)


if __name__ == "__main__":
    import reference
    inputs = {k: np.asarray(v) for k, v in reference.setup_inputs().items()}
    got = kernel(**inputs)
    print(got.shape, got.dtype)


# revision 29
# speedup vs baseline: 2.0068x; 1.0374x over previous
"""Trainium2 Bass kernel for nn_DeepConvGraphEncoderPre.

Model: 4x GCN (dense normalized adjacency) -> mean-pool over nodes ->
single-step BiLSTM -> fc -> temporal attention over T -> linear head.

Sharding: data-parallel over batch B=8 across 8 NeuronCores (1 batch row
per core).  The normalized dense adjacency A^T [256,512-layout] is built
ON DEVICE from edge_index via one-hot matmuls (exact, handles duplicate
edges); self-loops are added analytically as an identity.  Every GCN
layer is two dense matmuls (aggregate-first): x <- relu((A x) W + b).

Key optimizations vs the f32r baseline:
- all GCN matmuls in bf16 (validated: final rel err ~3e-3 vs 2e-2 tol);
  every matmul streams at 1 cycle/row regardless of moving-free size.
- graph PAIRS merged into single matmuls for L1/L2 via block-diagonal
  W1/W2 (built on host), halving matmul count there.
- all weights are pre-laid-out and pre-cast on HOST (pure relayout);
  input data is host-transposed into the exact SBUF layout so the big
  DMA is 128 contiguous 8KB lines instead of 8192 x 256B descriptors.
- PSUM evacuations balanced across vector+scalar; node-pooling fused
  into relu via tensor_tensor_reduce on vector.
- LSTM tail: forget gate dropped (unused at window_size=1), sigmoid
  computed from tanh (host-folded 1/2 scales) so one activation-table
  load covers i/o/g/c; attention bias dropped (softmax shift-invariant);
  weighted sum via fused multiply-accumulate instead of extra matmuls.
"""

import numpy as np
import ml_dtypes

B, T, N, F, E = 8, 32, 256, 64, 4096
H, EMB, OUT = 256, 256, 512
NCORES = 8
NPAIR = T // 2  # graph pairs per core

_CACHE = {}
RUN_KWARGS = {}   # test harness may set {"trace": True, ...}
LAST_RESULT = None


def _build(flags):
    import concourse.mybir as mybir
    import concourse.tile as tile
    from concourse import bacc
    from concourse.masks import make_identity

    dt = mybir.dt
    f32, f32r, bf16, i32 = dt.float32, dt.float32r, dt.bfloat16, dt.int32
    AF = mybir.ActivationFunctionType
    ALU = mybir.AluOpType

    gcn_bias, lstm_bias, fc_bias, out_bias = (
        flags["gcn_bias"], flags["lstm_bias"], flags["fc_bias"], flags["out_bias"])

    nc = bacc.Bacc("TRN2", target_bir_lowering=False, debug=False,
                   num_devices=NCORES)

    def rf(ap):
        return ap.bitcast(f32r)

    # ---------------- DRAM I/O (all host-prepped layouts) ----------------
    x0_d = nc.dram_tensor("x0", [128, NPAIR * 256], bf16, kind="ExternalInput")
    edge_d = nc.dram_tensor("edge_index", [2, E], i32, kind="ExternalInput")
    W1_d = nc.dram_tensor("W1blk", [128, 128], bf16, kind="ExternalInput")
    W2_d = nc.dram_tensor("W2blk", [128, 256], bf16, kind="ExternalInput")
    W3_d = nc.dram_tensor("W3p", [128, 256], bf16, kind="ExternalInput")
    W4_d = nc.dram_tensor("W4p", [128, 512], bf16, kind="ExternalInput")
    WihT_d = nc.dram_tensor("WihTp", [128, 3072], f32r, kind="ExternalInput")
    fcW_d = nc.dram_tensor("fcWp", [128, 1024], f32r, kind="ExternalInput")
    attnW_d = nc.dram_tensor("attnWp", [128, 2], f32r, kind="ExternalInput")
    outW_d = nc.dram_tensor("outWp", [128, 1024], f32r, kind="ExternalInput")
    if gcn_bias:
        bb1_d = nc.dram_tensor("bb1", [128, 128], f32, kind="ExternalInput")
        bb2_d = nc.dram_tensor("bb2", [128, 256], f32, kind="ExternalInput")
        bb3_d = nc.dram_tensor("bb3", [128, 256], f32, kind="ExternalInput")
        b4c_d = nc.dram_tensor("b4col", [128, 2], f32, kind="ExternalInput")
    if lstm_bias:
        bihT_d = nc.dram_tensor("bihT", [1, 1536], f32r, kind="ExternalInput")
    if fc_bias:
        fcb_d = nc.dram_tensor("fcb_col", [128, 2], f32, kind="ExternalInput")
    if out_bias:
        outb_d = nc.dram_tensor("outb_row", [1, 512], f32r, kind="ExternalInput")
    out_d = nc.dram_tensor("out", [1, OUT], f32, kind="ExternalOutput")

    with tile.TileContext(nc) as tc:
        with tc.tile_pool(name="const", bufs=1) as cp:
            AT_sb = cp.tile([128, 512], bf16)       # col k*256+d ; A^T[s,d], s=k*128+p
            x0_sb = cp.tile([128, NPAIR * 256], bf16)
            W1_sb = cp.tile([128, 128], bf16)
            W2_sb = cp.tile([128, 256], bf16)
            W3_sb = cp.tile([128, 256], bf16)
            W4_sb = cp.tile([128, 512], bf16)
            WihT_sb = cp.tile([128, 3072], f32r)    # col k*1536 + g'*512 + d*256 + h
            fcW_sb = cp.tile([128, 1024], f32r)     # col k*256 + m   (pre-scaled 0.5)
            attnW_sb = cp.tile([128, 2], f32r)
            outW_sb = cp.tile([128, 1024], f32r)    # col mo*512 + o
            pooledT_sb = cp.tile([128, 64], f32r)   # col mo*32 + t
            ident = cp.tile([128, 128], f32)
            ones_col = cp.tile([128, 1], f32)
            ones_row = cp.tile([1, 128], f32)
            ones2c = cp.tile([128, 256], f32)
            if gcn_bias:
                bb1_sb = cp.tile([128, 128], f32)
                bb2_sb = cp.tile([128, 256], f32)
                bb3_sb = cp.tile([128, 256], f32)
                b4c_sb = cp.tile([128, 2], f32)
            if lstm_bias or out_bias:
                ones_f = cp.tile([1, 32], f32)
                ones_r = cp.tile([1, 32], f32r)
            if lstm_bias:
                bihT_sb = cp.tile([1, 1536], f32r)
            if fc_bias:
                fcb_sb = cp.tile([128, 2], f32)
            if out_bias:
                outb_sb = cp.tile([1, 512], f32r)

            # ---- DMA issue: sync gets the critical-path tensors, scalar
            # the weights (parallel issue on two queues) ----
            eg_sb = cp.tile([128, 64], i32)   # col j<32: src ; col 32+j: dst
            nc.sync.dma_start(
                out=eg_sb[:].rearrange("p (two j) -> p two j", two=2),
                in_=edge_d.ap().rearrange("two (p j) -> p two j", p=128))
            nc.sync.dma_start(out=x0_sb[:], in_=x0_d.ap())
            nc.sync.dma_start(out=WihT_sb[:], in_=WihT_d.ap())
            nc.sync.dma_start(out=W1_sb[:], in_=W1_d.ap())
            nc.sync.dma_start(out=W2_sb[:], in_=W2_d.ap())
            nc.sync.dma_start(out=W3_sb[:], in_=W3_d.ap())
            nc.sync.dma_start(out=W4_sb[:], in_=W4_d.ap())
            nc.sync.dma_start(out=fcW_sb[:], in_=fcW_d.ap())
            nc.sync.dma_start(out=attnW_sb[:], in_=attnW_d.ap())
            nc.sync.dma_start(out=outW_sb[:], in_=outW_d.ap())
            if gcn_bias:
                nc.sync.dma_start(out=bb1_sb[:], in_=bb1_d.ap())
                nc.sync.dma_start(out=bb2_sb[:], in_=bb2_d.ap())
                nc.sync.dma_start(out=bb3_sb[:], in_=bb3_d.ap())
                nc.sync.dma_start(out=b4c_sb[:], in_=b4c_d.ap())
            if lstm_bias:
                nc.sync.dma_start(out=bihT_sb[:], in_=bihT_d.ap())
            if fc_bias:
                nc.sync.dma_start(out=fcb_sb[:], in_=fcb_d.ap())
            if out_bias:
                nc.sync.dma_start(out=outb_sb[:], in_=outb_d.ap())

            nc.gpsimd.memset(ones_col[:], 1.0)
            nc.gpsimd.memset(ones_row[:], 1.0)
            nc.gpsimd.memset(ones2c[:], 1.0)
            make_identity(nc, ident[:])
            if lstm_bias or out_bias:
                nc.gpsimd.memset(ones_f[:], 1.0)
                nc.vector.tensor_copy(ones_r[:], ones_f[:])

            # ============ stage A: A^T build ============
            with (
                tc.tile_pool(name="ab_sb", bufs=2) as ab,
                tc.tile_pool(name="ab_ps", bufs=1, space="PSUM") as abp,
                tc.tile_pool(name="oh", bufs=4) as ohp,
            ):
                # iota 0..255 on every partition (channel_multiplier=0), cast bf16
                iota_i = ab.tile([128, 256], i32)
                nc.gpsimd.iota(iota_i[:], pattern=[[1, 256]], base=0,
                               channel_multiplier=0)
                iota_bc = ab.tile([128, 256], bf16)
                nc.vector.tensor_copy(iota_bc[:], iota_i[:])
                eg_b = ab.tile([128, 64], f32)
                nc.vector.tensor_copy(eg_b[:], eg_sb[:])

                # accumulate A^T_unnorm = sum_e onehot_src^T(slice) @ onehot_dst
                # (src one-hots on vector, dst one-hots on gpsimd)
                atun_ps = abp.tile([128, 512], f32)
                for c in range(32):
                    oh_s = ohp.tile([128, 256], bf16, tag="ohs")
                    nc.vector.tensor_scalar(oh_s[:], iota_bc[:],
                                            eg_b[:, c:c + 1], None,
                                            op0=ALU.is_equal)
                    oh_d = ohp.tile([128, 256], bf16, tag="ohd")
                    nc.vector.tensor_scalar(oh_d[:], iota_bc[:],
                                            eg_b[:, 32 + c:33 + c], None,
                                            op0=ALU.is_equal)
                    for m in (0, 1):
                        nc.tensor.matmul(atun_ps[:, m * 256:(m + 1) * 256],
                                         oh_s[:, m * 128:(m + 1) * 128], oh_d[:],
                                         start=(c == 0 and m == 0),
                                         stop=(c == 31 and m == 1))
                atun_sb = ab.tile([128, 512], f32r)
                nc.vector.tensor_copy(atun_sb[:], atun_ps[:])
                # self-loops: += I on the diagonal (s = m*128+p, d = s)
                for m in (0, 1):
                    sl = atun_sb[:, m * 256 + m * 128: m * 256 + (m + 1) * 128]
                    nc.vector.tensor_add(sl, sl, ident[:])

                # deg (row + col forms), dinv = 1/sqrt(deg)   (deg >= 1 always)
                deg_ps = abp.tile([1, 256], f32, tag="deg")
                for m in (0, 1):
                    nc.tensor.matmul(deg_ps[:], ones_col[:],
                                     atun_sb[:, m * 256:(m + 1) * 256].bitcast(f32),
                                     start=(m == 0), stop=(m == 1))
                degc_ps = abp.tile([128, 2], f32, tag="degc")
                for dm in (0, 1):
                    for m in (0, 1):
                        nc.tensor.matmul(
                            degc_ps[:, dm:dm + 1],
                            atun_sb[:, m * 256 + dm * 128:
                                    m * 256 + (dm + 1) * 128].bitcast(f32),
                            ones_col[:], start=(m == 0), stop=(m == 1))
                dinv_row = ab.tile([1, 256], f32)
                nc.vector.reciprocal(dinv_row[:], deg_ps[:])
                nc.scalar.sqrt(dinv_row[:], dinv_row[:])
                dinv_col = ab.tile([128, 2], f32)
                nc.vector.reciprocal(dinv_col[:], degc_ps[:])
                nc.scalar.sqrt(dinv_col[:], dinv_col[:])
                # broadcast dinv to all partitions via rank-1 PE matmul
                dinv_bc = abp.tile([128, 256], f32, tag="dbc")
                nc.tensor.matmul(dinv_bc[:], ones_row[:], dinv_row[:],
                                 start=True, stop=True)

                # AT[s,d] = dinv[s] * ATun[s,d] * dinv[d]  (bf16 out)
                for m in (0, 1):
                    nc.vector.scalar_tensor_tensor(
                        out=AT_sb[:, m * 256:(m + 1) * 256],
                        in0=atun_sb[:, m * 256:(m + 1) * 256],
                        scalar=dinv_col[:, m:m + 1],
                        in1=dinv_bc[:],
                        op0=ALU.mult, op1=ALU.mult)

            # ================= stage B: GCN loop (graph pairs) =================
            with (
                tc.tile_pool(name="work", bufs=2) as wk,
                tc.tile_pool(name="psA", bufs=2, space="PSUM") as psA,
                tc.tile_pool(name="psB", bufs=2, space="PSUM") as psB,
                tc.tile_pool(name="psC", bufs=2, space="PSUM") as psC,
            ):
                # Two pairs in flight (software pipelining): each stage is
                # emitted for both pairs back-to-back so the cross-engine
                # dependency latency of one pair hides under the other's work.
                tl_ = {}

                def s_l1a(j):
                    agg1 = psC.tile([128, 256], f32, tag="C")
                    tl_[j, "agg1"] = agg1
                    for k in (0, 1):
                        nc.tensor.matmul(
                            agg1[:],
                            x0_sb[:, j * 256 + k * 128: j * 256 + (k + 1) * 128],
                            AT_sb[:, k * 256:(k + 1) * 256],
                            start=(k == 0), stop=(k == 1))

                def s_l1ev(j):
                    agg1_sb = wk.tile([128, 256], bf16, tag="agg1")
                    tl_[j, "agg1_sb"] = agg1_sb
                    nc.vector.tensor_copy(agg1_sb[:], tl_[j, "agg1"][:])

                def s_l1w(j):
                    z1 = psC.tile([128, 256], f32, tag="C")
                    tl_[j, "z1"] = z1
                    agg1_sb = tl_[j, "agg1_sb"]
                    for m in (0, 1):
                        nc.tensor.matmul(z1[:, m * 128:(m + 1) * 128],
                                         agg1_sb[:, m * 128:(m + 1) * 128],
                                         W1_sb[:], start=True, stop=True)

                def s_x1(j):
                    z1 = tl_[j, "z1"]
                    x1 = wk.tile([128, 256], bf16, tag="x1")
                    tl_[j, "x1"] = x1
                    if gcn_bias:
                        z1b = wk.tile([128, 256], f32, tag="z1b")
                        nc.vector.tensor_add(
                            z1b[:].rearrange("p (m q) -> p m q", m=2),
                            z1[:].rearrange("p (m q) -> p m q", m=2),
                            bb1_sb[:].rearrange("p q -> p 1 q").broadcast_to([128, 2, 128]))
                        nc.scalar.activation(x1[:], z1b[:], AF.Relu)
                    else:
                        nc.scalar.activation(x1[:], z1[:], AF.Relu)

                def s_l2a(j):
                    agg2 = psC.tile([128, 256], f32, tag="C")
                    tl_[j, "agg2"] = agg2
                    x1 = tl_[j, "x1"]
                    for k in (0, 1):
                        nc.tensor.matmul(agg2[:],
                                         x1[:, k * 128:(k + 1) * 128],
                                         AT_sb[:, k * 256:(k + 1) * 256],
                                         start=(k == 0), stop=(k == 1))

                def s_l2ev(j):
                    agg2_sb = wk.tile([128, 256], bf16, tag="agg2")
                    tl_[j, "agg2_sb"] = agg2_sb
                    nc.vector.tensor_copy(agg2_sb[:], tl_[j, "agg2"][:])

                def s_l2w(j):
                    z2 = psB.tile([128, 512], f32, tag="B")
                    tl_[j, "z2"] = z2
                    agg2_sb = tl_[j, "agg2_sb"]
                    for m in (0, 1):
                        nc.tensor.matmul(z2[:, m * 256:(m + 1) * 256],
                                         agg2_sb[:, m * 128:(m + 1) * 128],
                                         W2_sb[:], start=True, stop=True)

                def s_x2(j):
                    z2 = tl_[j, "z2"]
                    x2 = wk.tile([128, 512], bf16, tag="x2")
                    tl_[j, "x2"] = x2
                    if gcn_bias:
                        z2b = wk.tile([128, 512], f32, tag="z2b")
                        nc.vector.tensor_add(
                            z2b[:].rearrange("p (m q) -> p m q", m=2),
                            z2[:].rearrange("p (m q) -> p m q", m=2),
                            bb2_sb[:].rearrange("p q -> p 1 q").broadcast_to([128, 2, 256]))
                        nc.scalar.activation(x2[:], z2b[:], AF.Relu)
                    else:
                        nc.scalar.activation(x2[:], z2[:], AF.Relu)

                def s_l3a(j):
                    agg3 = psB.tile([128, 512], f32, tag="B")
                    tl_[j, "agg3"] = agg3
                    x2 = tl_[j, "x2"]
                    for g in (0, 1):
                        for k in (0, 1):
                            nc.tensor.matmul(
                                agg3[:, g * 256:(g + 1) * 256],
                                x2[:, k * 256 + g * 128: k * 256 + (g + 1) * 128],
                                AT_sb[:, k * 256:(k + 1) * 256],
                                start=(k == 0), stop=(k == 1))

                def s_l3ev(j):
                    agg3_sb = wk.tile([128, 512], bf16, tag="agg3")
                    tl_[j, "agg3_sb"] = agg3_sb
                    nc.vector.tensor_copy(agg3_sb[:], tl_[j, "agg3"][:])

                def s_l3w(j):
                    z3 = psA.tile([128, 1024], f32, tag="A")
                    tl_[j, "z3"] = z3
                    agg3_sb = tl_[j, "agg3_sb"]
                    for g in (0, 1):
                        for m in (0, 1):
                            nc.tensor.matmul(
                                z3[:, g * 512 + m * 256: g * 512 + (m + 1) * 256],
                                agg3_sb[:, g * 256 + m * 128: g * 256 + (m + 1) * 128],
                                W3_sb[:], start=True, stop=True)

                def s_x3(j):
                    z3 = tl_[j, "z3"]
                    x3 = wk.tile([128, 1024], bf16, tag="x3")
                    tl_[j, "x3"] = x3
                    if gcn_bias:
                        z3b = wk.tile([128, 1024], f32, tag="z3b")
                        nc.vector.tensor_add(
                            z3b[:].rearrange("p (gm q) -> p gm q", gm=4),
                            z3[:].rearrange("p (gm q) -> p gm q", gm=4),
                            bb3_sb[:].rearrange("p q -> p 1 q").broadcast_to([128, 4, 256]))
                        nc.scalar.activation(x3[:], z3b[:], AF.Relu)
                    else:
                        nc.scalar.activation(x3[:], z3[:], AF.Relu)

                def s_l4a(j):
                    agg4 = psA.tile([128, 1024], f32, tag="A")
                    tl_[j, "agg4"] = agg4
                    x3 = tl_[j, "x3"]
                    for g in (0, 1):
                        for mc in (0, 1):
                            for k in (0, 1):
                                nc.tensor.matmul(
                                    agg4[:, g * 512 + mc * 256: g * 512 + (mc + 1) * 256],
                                    x3[:, g * 512 + k * 256 + mc * 128:
                                          g * 512 + k * 256 + (mc + 1) * 128],
                                    AT_sb[:, k * 256:(k + 1) * 256],
                                    start=(k == 0), stop=(k == 1))

                def s_l4ev(j):
                    # split the big evacuation across vector (g=0) and scalar
                    # (g=1) so neither engine eats the full 1024 columns
                    agg4 = tl_[j, "agg4"]
                    a4a = wk.tile([128, 512], bf16, tag="agg4a")
                    a4b = wk.tile([128, 512], bf16, tag="agg4b")
                    tl_[j, "a4"] = (a4a, a4b)
                    nc.vector.tensor_copy(a4a[:], agg4[:, 0:512])
                    nc.scalar.copy(a4b[:], agg4[:, 512:1024])

                def s_l4w(j):
                    z4 = psA.tile([128, 1024], f32, tag="A")
                    tl_[j, "z4"] = z4
                    a4 = tl_[j, "a4"]
                    for g in (0, 1):
                        for mo in (0, 1):
                            for k in (0, 1):
                                nc.tensor.matmul(
                                    z4[:, g * 512 + mo * 256: g * 512 + (mo + 1) * 256],
                                    W4_sb[:, k * 256 + mo * 128: k * 256 + (mo + 1) * 128],
                                    a4[g][:, k * 256:(k + 1) * 256],
                                    start=(k == 0), stop=(k == 1))

                def s_pool(j):
                    # fused relu + node-sum via STT accumulate (1/N in WihT);
                    # mo outer so the LSTM's k=0 gate matmuls (which only read
                    # pooledT cols 0:32 = mo 0) unblock before mo=1 drains
                    z4 = tl_[j, "z4"]
                    x4 = wk.tile([128, 1024], bf16, tag="x4")
                    if gcn_bias:
                        for g in (0, 1):
                            for mo in (0, 1):
                                sl = slice(g * 512 + mo * 256, g * 512 + (mo + 1) * 256)
                                nc.scalar.activation(
                                    x4[:, sl], z4[:, sl], AF.Relu,
                                    bias=b4c_sb[:, mo:mo + 1])
                        with nc.allow_low_precision(reason="f32r pool accum"):
                            for mo in (0, 1):
                                for g in (0, 1):
                                    sl = slice(g * 512 + mo * 256,
                                               g * 512 + (mo + 1) * 256)
                                    nc.vector.tensor_reduce(
                                        out=pooledT_sb[:, mo * 32 + 2 * j + g:
                                                       mo * 32 + 2 * j + g + 1],
                                        in_=x4[:, sl],
                                        axis=mybir.AxisListType.X, op=ALU.add)
                    else:
                        with nc.allow_low_precision(reason="f32r pool accum"):
                            for mo in (0, 1):
                                for g in (0, 1):
                                    sl = slice(g * 512 + mo * 256,
                                               g * 512 + (mo + 1) * 256)
                                    nc.vector.scalar_tensor_tensor(
                                        out=x4[:, sl], in0=z4[:, sl], scalar=0.0,
                                        in1=ones2c[:], op0=ALU.max, op1=ALU.mult,
                                        accum_out=pooledT_sb[:, mo * 32 + 2 * j + g:
                                                             mo * 32 + 2 * j + g + 1])

                # software pipeline across 2-pair groups: the front half of
                # group G runs while the back half of group G-1 drains, so the
                # tensor engine never waits for the pool/evac chain
                HEAD = [s_l1a, s_l1ev, s_l1w, s_x1, s_l2a, s_l2ev, s_l2w, s_x2]
                TAIL = [s_l3a, s_l3ev, s_l3w, s_x3, s_l4a, s_l4ev, s_l4w, s_pool]
                prev = None
                for base in range(0, NPAIR, 2):
                    for i in range(8):
                        if prev is not None:
                            TAIL[i](prev)
                            TAIL[i](prev + 1)
                        HEAD[i](base)
                        HEAD[i](base + 1)
                    prev = base
                for i in range(8):
                    TAIL[i](prev)
                    TAIL[i](prev + 1)

            # ======= stage C: LSTM + fc + attention + head =======
            # gates laid out (i, o, g) x (dir) x h; i,o have 0.5 folded into
            # WihT so sigmoid(x) = 0.5*(1+tanh(x/2)) needs only tanh.
            with (
                tc.tile_pool(name="tail", bufs=1) as tl,
                tc.tile_pool(name="tailps_g", bufs=1, space="PSUM") as tpg,
                tc.tile_pool(name="tailps", bufs=2, space="PSUM") as tp,
            ):
                # preload the tanh activation table while the last GCN pairs
                # finish (dummy op on a const tile; scalar is idle here)
                scrap = tl.tile([1, 16], f32)
                nc.scalar.activation(scrap[:], ones_row[0:1, 0:16], AF.Tanh)

                g_ps = tpg.tile([32, 1536], f32, tag="gates")
                for s in range(3):
                    for k in (0, 1):
                        nc.tensor.matmul(
                            g_ps[:, s * 512:(s + 1) * 512],
                            rf(pooledT_sb[:, k * 32:(k + 1) * 32]),
                            WihT_sb[:, k * 1536 + s * 512: k * 1536 + (s + 1) * 512],
                            start=(k == 0),
                            stop=(k == 1 and not lstm_bias))
                    if lstm_bias:
                        nc.tensor.matmul(g_ps[:, s * 512:(s + 1) * 512],
                                         ones_r[:],
                                         bihT_sb[:, s * 512:(s + 1) * 512],
                                         start=False, stop=True)
                th = tl.tile([32, 1536], f32)
                nc.scalar.activation(th[:], g_ps[:], AF.Tanh)
                # c2 = 2c = (1+tanh(i/2))*tanh(g) ; tc = tanh(c2 * 0.5)
                c2 = tl.tile([32, 512], f32)
                nc.vector.scalar_tensor_tensor(
                    out=c2[:], in0=th[:, 0:512], scalar=1.0,
                    in1=th[:, 1024:1536], op0=ALU.add, op1=ALU.mult)
                tc_sb = tl.tile([32, 512], f32)
                nc.scalar.activation(tc_sb[:], c2[:], AF.Tanh, scale=0.5)
                # h2 = 2h = (1+tanh(o/2))*tanh(c); the remaining 1/2 is folded
                # into fcW (pre-scaled 0.5 on host)
                h2 = tl.tile([32, 512], f32)
                nc.vector.scalar_tensor_tensor(
                    out=h2[:], in0=th[:, 512:1024], scalar=1.0,
                    in1=tc_sb[:], op0=ALU.add, op1=ALU.mult)

                # transpose h2 -> hT [128, (k,t)]
                hT_ps = tp.tile([128, 128], f32, tag="small")
                for k in range(4):
                    nc.tensor.transpose(hT_ps[:, k * 32:(k + 1) * 32],
                                        h2[:, k * 128:(k + 1) * 128],
                                        ident[0:32, 0:32])
                hT_sb = tl.tile([128, 128], f32r)
                nc.vector.tensor_copy(hT_sb[:], hT_ps[:])

                # embT [128, (mo,t)] = fcW^T @ hT
                embT_ps = tp.tile([128, 64], f32, tag="small")
                for mo in (0, 1):
                    for k in range(4):
                        nc.tensor.matmul(
                            embT_ps[:, mo * 32:(mo + 1) * 32],
                            fcW_sb[:, k * 256 + mo * 128: k * 256 + (mo + 1) * 128],
                            hT_sb[:, k * 32:(k + 1) * 32],
                            start=(k == 0), stop=(k == 3))
                embT_sb = tl.tile([128, 64], f32r)
                if fc_bias:
                    for mo in (0, 1):
                        nc.scalar.activation(embT_sb[:, mo * 32:(mo + 1) * 32],
                                             embT_ps[:, mo * 32:(mo + 1) * 32],
                                             AF.Identity,
                                             bias=fcb_sb[:, mo:mo + 1])
                else:
                    nc.vector.tensor_copy(embT_sb[:], embT_ps[:])

                # attention scores [1, 32]; attn_b dropped (softmax shift-inv);
                # no max-subtract (scores are O(1) by construction)
                sc_ps = tp.tile([1, 32], f32, tag="small")
                for mo in (0, 1):
                    nc.tensor.matmul(sc_ps[:], attnW_sb[:, mo:mo + 1],
                                     embT_sb[:, mo * 32:(mo + 1) * 32],
                                     start=(mo == 0), stop=(mo == 1))
                ex = tl.tile([1, 32], f32)
                ssum = tl.tile([1, 1], f32)
                nc.scalar.activation(ex[:], sc_ps[:], AF.Exp, accum_out=ssum[:])
                rs = tl.tile([1, 1], f32)
                nc.vector.reciprocal(rs[:], ssum[:])
                w_row = tl.tile([1, 32], f32)
                nc.vector.tensor_scalar_mul(w_row[:], ex[:], rs[:])
                w_bc = tp.tile([128, 32], f32, tag="wbc")
                nc.tensor.matmul(w_bc[:], ones_row[:], w_row[:],
                                 start=True, stop=True)

                # x_weighted[m] = sum_t embT[m,t] * w[t]  (fused mul+accum)
                xw_scr = tl.tile([128, 64], f32r)
                xw_col = tl.tile([128, 2], f32r)
                with nc.allow_low_precision(reason="f32r weighted-sum accum"):
                    for mo in (0, 1):
                        nc.vector.scalar_tensor_tensor(
                            out=xw_scr[:, mo * 32:(mo + 1) * 32],
                            in0=embT_sb[:, mo * 32:(mo + 1) * 32], scalar=1.0,
                            in1=w_bc[:], op0=ALU.mult, op1=ALU.mult,
                            accum_out=xw_col[:, mo:mo + 1])

                # head: out = xw @ out_W (+ out_b)
                fin_ps = tp.tile([1, 512], f32, tag="small")
                for mo in (0, 1):
                    nc.tensor.matmul(fin_ps[:], xw_col[:, mo:mo + 1],
                                     outW_sb[:, mo * 512:(mo + 1) * 512],
                                     start=(mo == 0),
                                     stop=(mo == 1 and not out_bias))
                if out_bias:
                    nc.tensor.matmul(fin_ps[:], ones_r[0:1, 0:1], outb_sb[:],
                                     start=False, stop=True)
                fin_sb = tl.tile([1, 512], f32)
                nc.vector.tensor_copy(fin_sb[:], fin_ps[:])
                nc.sync.dma_start(out=out_d.ap(), in_=fin_sb[:])

    nc.compile()
    return nc


def _get_nc(flags):
    key = tuple(sorted(flags.items()))
    if key not in _CACHE:
        _CACHE[key] = _build(flags)
    return _CACHE[key]


def kernel(**inputs):
    from concourse import bass_utils

    bf = ml_dtypes.bfloat16
    inp = {k: np.asarray(v) for k, v in inputs.items()}
    flags = {
        "gcn_bias": any(np.any(inp[f"b{i}"]) for i in (1, 2, 3, 4)),
        "lstm_bias": any(np.any(inp[k]) for k in
                         ("b_ih_f", "b_hh_f", "b_ih_b", "b_hh_b")),
        "fc_bias": bool(np.any(inp["fc_b"])),
        "out_bias": bool(np.any(inp["out_b"])),
    }
    nc = _get_nc(flags)

    f32 = np.float32
    W1 = inp["W1"].astype(f32)
    W2 = inp["W2"].astype(f32)
    W1blk = np.zeros((128, 128), f32)
    W1blk[:64, :64] = W1
    W1blk[64:, 64:] = W1
    W2blk = np.zeros((128, 256), f32)
    W2blk[:64, :128] = W2
    W2blk[64:, 128:] = W2
    W4p = inp["W4"].astype(f32).reshape(2, 128, 256).transpose(1, 0, 2).reshape(128, 512)

    # WihT: [co, g'*512 + d*256 + h], gate order (i, o, g); i,o scaled 0.5
    # (sigmoid-from-tanh), everything scaled 1/N (mean-pool folded in)
    M = np.zeros((256, 1536), f32)
    for di, dname in enumerate(("f", "b")):
        Wih = inp[f"W_ih_{dname}"].astype(f32)  # [4H, H] rows gate*256+h
        for gdst, (gsrc, sc) in enumerate([(0, 0.5), (3, 0.5), (2, 1.0)]):
            M[:, gdst * 512 + di * 256: gdst * 512 + (di + 1) * 256] = \
                Wih[gsrc * 256:(gsrc + 1) * 256, :].T * (sc / N)
    WihTp = M.reshape(2, 128, 1536).transpose(1, 0, 2).reshape(128, 3072)

    fcWp = (inp["fc_W"].astype(f32) * 0.5).reshape(4, 128, 256) \
        .transpose(1, 0, 2).reshape(128, 1024)
    attnWp = np.ascontiguousarray(inp["attn_W"].astype(f32).reshape(2, 128).T)
    outWp = inp["out_W"].astype(f32).reshape(2, 128, 512) \
        .transpose(1, 0, 2).reshape(128, 1024)

    base = {
        "edge_index": np.ascontiguousarray(inp["edge_index"].astype(np.int32)),
        "W1blk": np.ascontiguousarray(W1blk.astype(bf)),
        "W2blk": np.ascontiguousarray(W2blk.astype(bf)),
        "W3p": np.ascontiguousarray(inp["W3"].astype(f32).astype(bf)),
        "W4p": np.ascontiguousarray(W4p.astype(bf)),
        "WihTp": np.ascontiguousarray(WihTp),
        "fcWp": np.ascontiguousarray(fcWp),
        "attnWp": attnWp,
        "outWp": np.ascontiguousarray(outWp),
    }
    if flags["gcn_bias"]:
        b1 = inp["b1"].astype(f32)
        b2 = inp["b2"].astype(f32)
        b3 = inp["b3"].astype(f32)
        b4 = inp["b4"].astype(f32)
        base["bb1"] = np.ascontiguousarray(
            np.tile(np.concatenate([b1, b1]), (128, 1)))
        base["bb2"] = np.ascontiguousarray(
            np.tile(np.concatenate([b2, b2]), (128, 1)))
        base["bb3"] = np.ascontiguousarray(np.tile(b3, (128, 1)))
        base["b4col"] = np.ascontiguousarray(b4.reshape(2, 128).T)
    if flags["lstm_bias"]:
        bihT = np.zeros((1, 1536), f32)
        for di, dname in enumerate(("f", "b")):
            bsum = (inp[f"b_ih_{dname}"] + inp[f"b_hh_{dname}"]).astype(f32)
            for gdst, (gsrc, sc) in enumerate([(0, 0.5), (3, 0.5), (2, 1.0)]):
                bihT[0, gdst * 512 + di * 256: gdst * 512 + (di + 1) * 256] = \
                    bsum[gsrc * 256:(gsrc + 1) * 256] * sc
        base["bihT"] = bihT
    if flags["fc_bias"]:
        base["fcb_col"] = np.ascontiguousarray(
            inp["fc_b"].astype(f32).reshape(2, 128).T)
    if flags["out_bias"]:
        base["outb_row"] = np.ascontiguousarray(
            inp["out_b"].astype(f32).reshape(1, 512))

    # x0: [p, j*256 + k*128 + g*64 + c] = data[2j+g, k*128+p, c], bf16
    data = inp["data"].astype(f32)
    in_maps = []
    for c in range(NCORES):
        v = data[c].reshape(NPAIR, 2, 2, 128, F)          # [j, g, k, p, c]
        x0 = v.transpose(3, 0, 2, 1, 4).reshape(128, NPAIR * 256)
        in_maps.append(dict(base, x0=np.ascontiguousarray(x0.astype(bf))))

    global LAST_RESULT
    res = bass_utils.run_bass_kernel_spmd(nc, in_maps,
                                          core_ids=list(range(NCORES)),
                                          **RUN_KWARGS)
    LAST_RESULT = res
    return np.concatenate([r["out"] for r in res.results], axis=0)


if __name__ == "__main__":
    import reference
    inputs = {k: np.asarray(v) for k, v in reference.setup_inputs().items()}
    got = kernel(**inputs)
    print(got.shape, got.dtype)
